# revision 10
# baseline (speedup 1.0000x reference)
"""Trainium2 Bass kernel for nn_Attention_55894704390617.

Dense transformer attention block:
  xn = LN(x) ; q,k,v = xn @ wq/wk/wv ; q,k = headLN(q),headLN(k)
  out = softmax(q k^T / sqrt(dh)) v @ wo

Sharding over 8 NeuronCores: 2 (batch) x 4 (head groups of 8 heads).
Each core computes a partial output (its head-group's contribution to
out = attn_out @ wo); the host sums the 4 partials per batch.

Per-core data flow (matmuls in bf16, fp32 PSUM accumulation):
  - host pre-folds norm_w into wq/wk/wv and ships the transposed x and
    the -colsum(w) correction rows; LN mean is folded into each
    projection as a K=1 accumulation row (mu[s] x negc[m]); x-rstd is
    applied only to V (head-LN on Q/K is scale-invariant, so their
    x-rstd cancels)
  - all transposes (kT, qT, avT) are DMA xbar transposes issued from
    the otherwise-idle SP engine: no PE transposes, no PSUM staging
  - scores computed transposed (scoresT[j,i]) two heads per PSUM pair
    [128,1024]; one exp per pair on ACT (phase B ACT does only exp)
  - attn@V uses attn tiles as the stationary operand and V (+ones col
    for the softmax denominator) as the 65-column moving operand:
    cost-model matmul time scales with moving columns only
  - AV lands natural [i, dh]; denominator normalize is a per-partition
    scalar multiply, then a DMA transpose produces avT for the
    out-projection
  - head-LN apply runs on GPSIMD; bn stats on DVE read PSUM directly
  - out partials are written in bf16
"""

import numpy as np

S = 2048          # sequence length
D = 2048          # model dim
H_LOC = 8         # heads per core
DH = 64           # head dim
M_LOC = H_LOC * DH  # 512 inner dim per core
N_D = D // 128    # 16 d-tiles
N_S = S // 128    # 16 s-tiles
N_SC = S // 512   # 4 512-chunks
N_MT = M_LOC // 128  # 4 m-tiles per core
EPS = 1e-5

_COMPILED = {}


def _build():
    from concourse._compat import axon_active
    axon_active()
    import concourse.bacc as bacc
    import concourse.mybir as mybir
    import concourse.tile as tile
    from contextlib import ExitStack

    F32 = mybir.dt.float32
    BF16 = mybir.dt.bfloat16
    AF = mybir.ActivationFunctionType
    OP = mybir.AluOpType

    nc = bacc.Bacc(None, target_bir_lowering=False)

    x_nat = nc.dram_tensor("x_nat", [S, D], BF16, kind="ExternalInput")
    x_tr = nc.dram_tensor("x_tr", [D, S], BF16, kind="ExternalInput")
    wgq = nc.dram_tensor("wgq", [D, M_LOC], BF16, kind="ExternalInput")
    wgk = nc.dram_tensor("wgk", [D, M_LOC], BF16, kind="ExternalInput")
    wgv = nc.dram_tensor("wgv", [D, M_LOC], BF16, kind="ExternalInput")
    negc = nc.dram_tensor("negc", [3, M_LOC], BF16, kind="ExternalInput")
    wo = nc.dram_tensor("wo", [M_LOC, D], BF16, kind="ExternalInput")
    qn_w = nc.dram_tensor("qn_w", [DH], F32, kind="ExternalInput")
    kn_w = nc.dram_tensor("kn_w", [DH], F32, kind="ExternalInput")
    out = nc.dram_tensor("out", [S, D], BF16, kind="ExternalOutput")

    import os
    DBG = bool(os.environ.get("ATTN_DEBUG"))
    dbg = {}
    if DBG:
        dbg["kT"] = nc.dram_tensor("dbg_kT", [128, N_MT, S], BF16,
                                   kind="ExternalOutput")
        dbg["vext0"] = nc.dram_tensor("dbg_vext0", [128, H_LOC, DH + 1], BF16,
                                      kind="ExternalOutput")
        dbg["qT0"] = nc.dram_tensor("dbg_qT0", [128, N_MT, 512], BF16,
                                    kind="ExternalOutput")
        dbg["avT0"] = nc.dram_tensor("dbg_avT0", [128, N_MT, 512], BF16,
                                     kind="ExternalOutput")
        dbg["attn00"] = nc.dram_tensor("dbg_attn00", [128, 1024], BF16,
                                       kind="ExternalOutput")

    with tile.TileContext(nc) as tc:
        es = ExitStack()
        # ---- pools alive for the whole kernel ----
        consts = es.enter_context(tc.tile_pool(name="consts", bufs=1))
        dram = es.enter_context(tc.tile_pool(name="dram", bufs=1, space="DRAM"))
        xt_pool = es.enter_context(tc.tile_pool(name="xt", bufs=1))
        wgq_pool = es.enter_context(tc.tile_pool(name="wgq", bufs=1))
        kT_pool = es.enter_context(tc.tile_pool(name="kT", bufs=1))
        vext_pool = es.enter_context(tc.tile_pool(name="vext", bufs=1))

        eps_t = consts.tile([128, 1], F32, name="eps_t")
        nc.vector.memset(eps_t, EPS)

        # qn/kn replicated across partitions (f32 dma, then bf16 copy)
        from concourse.bass import AP
        rep_f32 = {}
        rep_bf = {}
        for nm, wten in (("q", qn_w), ("k", kn_w)):
            rf = consts.tile([128, H_LOC, DH], F32, name=f"{nm}n_repf")
            bsrc = AP(tensor=wten[:].tensor, offset=wten[:].offset,
                      ap=[[0, 128], [0, H_LOC], [1, DH]])
            nc.scalar.dma_start(out=rf, in_=bsrc)
            rb = consts.tile([128, H_LOC * DH], BF16, name=f"{nm}n_rep")
            nc.vector.tensor_copy(rb, rf.rearrange("p h d -> p (h d)"))
            rep_f32[nm] = rf
            rep_bf[nm] = rb

        negc_sb = [consts.tile([1, M_LOC], BF16, name=f"negc_sb{r}")
                   for r in range(3)]
        for r in range(3):
            nc.scalar.dma_start(out=negc_sb[r], in_=negc[r:r + 1, :])

        # per-s-tile stat tiles
        mu_col = [consts.tile([128, 1], F32, name=f"mu_col{t}") for t in range(N_S)]
        rstd_col = [consts.tile([128, 1], F32, name=f"rstd_col{t}")
                    for t in range(N_S)]
        mu_bf = [consts.tile([1, 128], BF16, name=f"mu_bf{t}") for t in range(N_S)]
        dscr = dram.tile([S], F32)

        xt = [xt_pool.tile([128, S], BF16, name=f"xt{t}") for t in range(N_D)]
        wg_q = [wgq_pool.tile([128, M_LOC], BF16, name=f"wg_q{t}")
                for t in range(N_D)]
        kT = kT_pool.tile([128, N_MT, S], BF16, name="kT")
        v_ext = [vext_pool.tile([128, H_LOC, DH + 1], BF16, name=f"vext{st}")
                 for st in range(N_S)]

        # ============ phase A: loads + stats + K,V projections ============
        with ExitStack() as ph:
            wg_pool = ph.enter_context(tc.tile_pool(name="wg", bufs=1))
            stage = ph.enter_context(tc.tile_pool(name="stage", bufs=3))
            scrA = ph.enter_context(tc.tile_pool(name="scrA", bufs=3))
            ps_mm = ph.enter_context(tc.tile_pool(name="ps_mm", bufs=4, space="PSUM"))

            wg = {"q": wg_q}
            for wname in ("k", "v"):
                wg[wname] = [wg_pool.tile([128, M_LOC], BF16, name=f"wg_{wname}{t}")
                             for t in range(N_D)]
            wdrams = {"q": wgq, "k": wgk, "v": wgv}
            NEGC_ROW = {"q": 0, "k": 1, "v": 2}

            def emit_stats(st):
                xst = stage.tile([128, S], BF16, tag="xst")
                nc.sync.dma_start(out=xst, in_=x_nat[st * 128:(st + 1) * 128, :])
                xg = xst.rearrange("p (n f) -> p n f", f=512)
                bn = scrA.tile([128, 4, 6], F32, tag="bn")
                for sg in range(4):
                    nc.vector.bn_stats(out=bn[:, sg, :], in_=xg[:, sg, :])
                mv = scrA.tile([128, 2], F32, tag="mv")
                nc.vector.bn_aggr(out=mv, in_=bn)
                nc.vector.tensor_copy(mu_col[st], mv[:, 0:1])
                # rstd = exp(-0.5*ln(var+eps)): Ln/Exp share one ACT
                # table with the softmax Exp, so no table reloads
                nc.scalar.activation(out=rstd_col[st], in_=mv[:, 1:2],
                                     func=AF.Ln, bias=eps_t, scale=1.0)
                nc.scalar.activation(out=rstd_col[st], in_=rstd_col[st],
                                     func=AF.Exp, scale=-0.5)
                nc.scalar.dma_start(
                    out=dscr[st * 128:(st + 1) * 128].rearrange(
                        "(p one) -> p one", one=1),
                    in_=mu_col[st])
                mur = scrA.tile([1, 128], F32, tag="mur")
                nc.scalar.dma_start(
                    out=mur,
                    in_=dscr[st * 128:(st + 1) * 128].rearrange(
                        "(one s) -> one s", one=1))
                nc.vector.tensor_copy(mu_bf[st], mur)

            # input DMAs: x on SP, weights on ACT (idle in phase A)
            for t in range(N_D):
                nc.scalar.dma_start(out=wg["k"][t],
                                    in_=wgk[t * 128:(t + 1) * 128, :])
                nc.sync.dma_start(out=xt[t], in_=x_tr[t * 128:(t + 1) * 128, :])
                nc.scalar.dma_start(out=wg["v"][t],
                                    in_=wgv[t * 128:(t + 1) * 128, :])
                if t == 0:
                    emit_stats(0)
            for st in range(1, N_S):
                emit_stats(st)
            # Q weights: land while K/V projections run
            for t in range(N_D):
                nc.scalar.dma_start(out=wg_q[t], in_=wgq[t * 128:(t + 1) * 128, :])

            def emit_headln_tail(p, dst_T, sl, nm):
                """Head-LN on PSUM proj result p, then DMA-transpose into
                dst_T[:, :, sl]. nm selects qn/kn."""
                pg = p.rearrange("p (h d) -> p h d", d=DH)
                bn8 = scrA.tile([128, H_LOC, 6], F32, tag="bn8")
                mv8 = scrA.tile([128, H_LOC, 2], F32, tag="mv8")
                for h in range(H_LOC):
                    nc.vector.bn_stats(out=bn8[:, h, :], in_=pg[:, h, :])
                    nc.vector.bn_aggr(out=mv8[:, h, :], in_=bn8[:, h, :])
                rstd8 = scrA.tile([128, H_LOC], F32, tag="rstd8")
                nc.scalar.activation(out=rstd8, in_=mv8[:, :, 1], func=AF.Ln,
                                     bias=eps_t, scale=1.0)
                nc.scalar.activation(out=rstd8, in_=rstd8, func=AF.Exp,
                                     scale=-0.5)
                lnb = scrA.tile([128, H_LOC, DH], BF16, tag="lnb")
                for h in range(H_LOC):
                    nc.vector.tensor_scalar(
                        out=lnb[:, h, :], in0=pg[:, h, :],
                        scalar1=mv8[:, h, 0:1], scalar2=rstd8[:, h:h + 1],
                        op0=OP.subtract, op1=OP.mult)
                lnb2 = scrA.tile([128, H_LOC * DH], BF16, tag="lnb2")
                nc.gpsimd.tensor_mul(out=lnb2,
                                     in0=lnb.rearrange("p h d -> p (h d)"),
                                     in1=rep_bf[nm])
                nc.sync.dma_start_transpose(out=dst_T[:, :, sl], in_=lnb2)

            for st in range(N_S):
                sl = slice(st * 128, (st + 1) * 128)
                # K projection + head-LN + transpose
                p = ps_mm.tile([128, M_LOC], F32, tag="mm", name=f"pk{st}")
                for t in range(N_D):
                    nc.tensor.matmul(p[:, :], xt[t][:, sl], wg["k"][t][:, :],
                                     start=(t == 0), stop=False)
                nc.tensor.matmul(p[:, :], mu_bf[st][:, :],
                                 negc_sb[NEGC_ROW["k"]][:, :],
                                 start=False, stop=True)
                emit_headln_tail(p, kT, sl, "k")
                # V projection + x-rstd + ones column
                p2 = ps_mm.tile([128, M_LOC], F32, tag="mm", name=f"pv{st}")
                for t in range(N_D):
                    nc.tensor.matmul(p2[:, :], xt[t][:, sl], wg["v"][t][:, :],
                                     start=(t == 0), stop=False)
                nc.tensor.matmul(p2[:, :], mu_bf[st][:, :],
                                 negc_sb[NEGC_ROW["v"]][:, :],
                                 start=False, stop=True)
                nc.vector.tensor_scalar_mul(
                    out=v_ext[st][:, :, 0:DH],
                    in0=p2.rearrange("p (h d) -> p h d", d=DH),
                    scalar1=rstd_col[st])
                nc.vector.memset(v_ext[st][:, :, DH:DH + 1], 1.0)

        if DBG:
            nc.sync.dma_start(out=dbg["kT"][:, :, :], in_=kT)
            nc.sync.dma_start(out=dbg["vext0"][:, :, :], in_=v_ext[0])

        # ============ phase B: Q proj + attention + out-proj ============
        with ExitStack() as ph:
            wo_pool = ph.enter_context(tc.tile_pool(name="wop", bufs=1))
            qT_pool = ph.enter_context(tc.tile_pool(name="qT", bufs=2))
            attn_pool = ph.enter_context(tc.tile_pool(name="attn", bufs=12))
            avsb_pool = ph.enter_context(tc.tile_pool(name="avsb", bufs=2))
            avT_pool = ph.enter_context(tc.tile_pool(name="avT", bufs=2))
            ot_pool = ph.enter_context(tc.tile_pool(name="ot", bufs=3))
            scrB = ph.enter_context(tc.tile_pool(name="scrB", bufs=3))
            # PSUM: sc 2x2 banks + av 2x1 + qp 1 + op 1 = 8 banks
            ps_sc = ph.enter_context(tc.tile_pool(name="ps_sc", bufs=2,
                                                  space="PSUM"))
            ps_av = ph.enter_context(tc.tile_pool(name="ps_av", bufs=2,
                                                  space="PSUM"))
            ps_qp = ph.enter_context(tc.tile_pool(name="ps_qp", bufs=1,
                                                  space="PSUM"))
            ps_op = ph.enter_context(tc.tile_pool(name="ps_op", bufs=1,
                                                  space="PSUM"))

            wo_sb = [wo_pool.tile([128, D], BF16, name=f"wo{mt}")
                     for mt in range(N_MT)]
            for mt in range(N_MT):
                nc.scalar.dma_start(out=wo_sb[mt],
                                    in_=wo[mt * 128:(mt + 1) * 128, :])

            def emit_qtail(st, p, qT_next):
                ssl = slice((st % 4) * 128, (st % 4 + 1) * 128)
                nc.tensor.matmul(p[:, :], mu_bf[st][:, :],
                                 negc_sb[NEGC_ROW["q"]][:, :],
                                 start=False, stop=True)
                pg = p.rearrange("p (h d) -> p h d", d=DH)
                bn8 = scrB.tile([128, H_LOC, 6], F32, tag="bn8")
                mv8 = scrB.tile([128, H_LOC, 2], F32, tag="mv8")
                for h in range(H_LOC):
                    nc.vector.bn_stats(out=bn8[:, h, :], in_=pg[:, h, :])
                    nc.vector.bn_aggr(out=mv8[:, h, :], in_=bn8[:, h, :])
                rstd8 = scrB.tile([128, H_LOC], F32, tag="rstd8")
                nc.scalar.activation(out=rstd8, in_=mv8[:, :, 1], func=AF.Ln,
                                     bias=eps_t, scale=1.0)
                nc.scalar.activation(out=rstd8, in_=rstd8, func=AF.Exp,
                                     scale=-0.5)
                lnb = scrB.tile([128, H_LOC, DH], BF16, tag="lnb")
                for h in range(H_LOC):
                    nc.vector.tensor_scalar(
                        out=lnb[:, h, :], in0=pg[:, h, :],
                        scalar1=mv8[:, h, 0:1], scalar2=rstd8[:, h:h + 1],
                        op0=OP.subtract, op1=OP.mult)
                lnb2 = scrB.tile([128, H_LOC * DH], BF16, tag="lnb2")
                nc.gpsimd.tensor_mul(out=lnb2,
                                     in0=lnb.rearrange("p h d -> p (h d)"),
                                     in1=rep_bf["q"])
                nc.sync.dma_start_transpose(out=qT_next[:, :, ssl], in_=lnb2)

            def emit_oproj(avT_src, st, do):
                sl = slice(st * 128, (st + 1) * 128)
                lsl = slice((st % 4) * 128, (st % 4 + 1) * 128)
                op = ps_op.tile([128, 512], F32, tag="op", name=f"op{st}_{do}")
                for mt in range(N_MT):
                    nc.tensor.matmul(op[:, :], avT_src[:, mt, lsl],
                                     wo_sb[mt][:, do * 512:(do + 1) * 512],
                                     start=(mt == 0), stop=(mt == N_MT - 1))
                ot = ot_pool.tile([128, 512], BF16, tag="ot")
                nc.vector.tensor_copy(ot, op)
                nc.gpsimd.dma_start(out=out[sl, do * 512:(do + 1) * 512],
                                    in_=ot)

            def alloc_qT(ic):
                return qT_pool.tile([128, N_MT, 512], BF16, tag="qT",
                                    name=f"qT{ic}")

            qT_cur = alloc_qT(0)
            for st in range(4):
                qp = ps_qp.tile([128, M_LOC], F32, tag="qp", name=f"qp{st}")
                qsl = slice(st * 128, (st + 1) * 128)
                for t in range(N_D):
                    nc.tensor.matmul(qp[:, :], xt[t][:, qsl], wg_q[t][:, :],
                                     start=(t == 0), stop=False)
                emit_qtail(st, qp, qT_cur)

            avT_prev = None
            for ic in range(N_SC):
                qT_next = alloc_qT(ic + 1) if ic + 1 < N_SC else None
                avT_cur = avT_pool.tile([128, N_MT, 512], BF16, tag="avT",
                                        name=f"avT{ic}")
                for hp in range(N_MT):
                    qst = 4 * (ic + 1) + hp if qT_next is not None else None
                    qp = None
                    if qst is not None:
                        qp = ps_qp.tile([128, M_LOC], F32, tag="qp",
                                        name=f"qp{qst}")
                    avsb = avsb_pool.tile([128, 4, 2, DH], BF16, tag="avsb",
                                          name=f"avsb{ic}_{hp}")

                    def emit_av_chain(attn_tiles, hs, it, h):
                        # one 16-step accumulation chain in its own PSUM bank
                        # (in-flight chains sharing a bank corrupt each other),
                        # then denominator normalize into avsb
                        av = ps_av.tile([128, DH + 1], F32, tag="av",
                                        name=f"av{ic}_{hp}_{hs}_{it}")
                        for pj in range(8):
                            for half in range(2):
                                csl = slice(half * 512 + it * 128,
                                            half * 512 + (it + 1) * 128)
                                nc.tensor.matmul(
                                    av[:, 0:DH + 1], attn_tiles[pj][:, csl],
                                    v_ext[2 * pj + half][:, h, :],
                                    start=(pj == 0 and half == 0),
                                    stop=(pj == 7 and half == 1))
                        nc.vector.reciprocal(av[:, DH:DH + 1], av[:, DH:DH + 1])
                        nc.vector.tensor_scalar_mul(
                            out=avsb[:, it, hs, :], in0=av[:, 0:DH],
                            scalar1=av[:, DH:DH + 1])

                    qp_step = [0]

                    def emit_qp_step():
                        if qp is not None and qp_step[0] < N_D:
                            t = qp_step[0]
                            qsl = slice(qst * 128, (qst + 1) * 128)
                            nc.tensor.matmul(qp[:, :], xt[t][:, qsl],
                                             wg_q[t][:, :],
                                             start=(t == 0), stop=False)
                            qp_step[0] += 1

                    for hs in range(2):
                        h = 2 * hp + hs
                        psl = slice(hs * DH, (hs + 1) * DH)
                        attn_tiles = []
                        for pj in range(8):
                            if avT_prev is not None and pj % 4 == 1:
                                emit_oproj(avT_prev, 4 * (ic - 1) + hp,
                                           2 * hs + pj // 4)
                            sc = ps_sc.tile([128, 1024], F32, tag="sc",
                                            name=f"sc{hs}_{pj}")
                            for half in range(2):
                                jt = 2 * pj + half
                                jsl = slice(jt * 128, (jt + 1) * 128)
                                nc.tensor.matmul(
                                    sc[:, half * 512:(half + 1) * 512],
                                    kT[psl, hp, jsl], qT_cur[psl, hp, :],
                                    start=True, stop=True)
                            a = attn_pool.tile([128, 1024], BF16, tag="attn")
                            nc.scalar.activation(out=a, in_=sc, func=AF.Exp,
                                                 scale=0.125)
                            if DBG and ic == 0 and hp == 0 and hs == 0 \
                                    and pj == 0:
                                nc.sync.dma_start(out=dbg["attn00"][:, :],
                                                  in_=a)
                            attn_tiles.append(a)
                            emit_qp_step()
                        # pass 1: chains for i-blocks 0,1 (banks A,B)
                        for it in range(2):
                            emit_av_chain(attn_tiles, hs, it, h)
                        # pass 2: chains for i-blocks 2,3 reuse the banks
                        for it in range(2, 4):
                            emit_av_chain(attn_tiles, hs, it, h)
                    nc.sync.dma_start_transpose(
                        out=avT_cur[:, hp, :].rearrange(
                            "p (t f) -> p t f", f=128),
                        in_=avsb.rearrange("p t h d -> p (t h d)"))
                    if qp is not None:
                        emit_qtail(qst, qp, qT_next)
                if DBG and ic == 0:
                    nc.sync.dma_start(out=dbg["qT0"][:, :, :], in_=qT_cur)
                    nc.sync.dma_start(out=dbg["avT0"][:, :, :], in_=avT_cur)
                avT_prev = avT_cur
                qT_cur = qT_next

            # tail: out-projection of the last chunk
            for st in range(4 * (N_SC - 1), 4 * N_SC):
                for do in range(N_SC):
                    emit_oproj(avT_prev, st, do)
        es.close()

    nc.compile()
    return nc


def _get_nc():
    if "nc" not in _COMPILED:
        _COMPILED["nc"] = _build()
    return _COMPILED["nc"]


def kernel(x, norm_w, wq, wk, wv, qn_w, kn_w, wo):
    import ml_dtypes
    from concourse.bass_utils import run_bass_kernel_spmd

    x = np.asarray(x, dtype=np.float32)
    norm_w = np.asarray(norm_w, dtype=np.float32)
    wq = np.asarray(wq, dtype=np.float32)
    wk = np.asarray(wk, dtype=np.float32)
    wv = np.asarray(wv, dtype=np.float32)
    qn_w = np.asarray(qn_w, dtype=np.float32)
    kn_w = np.asarray(kn_w, dtype=np.float32)
    wo = np.asarray(wo, dtype=np.float32)
    B = x.shape[0]

    nc = _get_nc()
    in_maps = []
    for c in range(8):
        b, g = c // 4, c % 4
        ms = slice(g * M_LOC, (g + 1) * M_LOC)
        gq = norm_w[:, None] * wq[:, ms]
        gk = norm_w[:, None] * wk[:, ms]
        gv = norm_w[:, None] * wv[:, ms]
        negc = -np.stack([gq.sum(0), gk.sum(0), gv.sum(0)])
        in_maps.append({
            "x_nat": np.ascontiguousarray(x[b]).astype(ml_dtypes.bfloat16),
            "x_tr": np.ascontiguousarray(x[b].T).astype(ml_dtypes.bfloat16),
            "wgq": np.ascontiguousarray(gq).astype(ml_dtypes.bfloat16),
            "wgk": np.ascontiguousarray(gk).astype(ml_dtypes.bfloat16),
            "wgv": np.ascontiguousarray(gv).astype(ml_dtypes.bfloat16),
            "negc": np.ascontiguousarray(negc).astype(ml_dtypes.bfloat16),
            "wo": np.ascontiguousarray(wo[ms, :]).astype(ml_dtypes.bfloat16),
            "qn_w": qn_w,
            "kn_w": kn_w,
        })
    res = run_bass_kernel_spmd(nc, in_maps, core_ids=list(range(8)))
    out = np.zeros((B, S, D), dtype=np.float32)
    for c in range(8):
        out[c // 4] += np.asarray(res.results[c]["out"], dtype=np.float32)
    return out


# revision 11
# speedup vs baseline: 1.0270x; 1.0270x over previous
"""Trainium2 Bass kernel for nn_Attention_55894704390617.

Dense transformer attention block:
  xn = LN(x) ; q,k,v = xn @ wq/wk/wv ; q,k = headLN(q),headLN(k)
  out = softmax(q k^T / sqrt(dh)) v @ wo

Sharding over 8 NeuronCores: 2 (batch) x 4 (head groups of 8 heads).
Each core computes a partial output (its head-group's contribution to
out = attn_out @ wo); the host sums the 4 partials per batch.

Per-core data flow (matmuls in bf16, fp32 PSUM accumulation):
  - host pre-folds norm_w into wq/wk/wv and ships the transposed x and
    the -colsum(w) correction rows; LN mean is folded into each
    projection as a K=1 accumulation row (mu[s] x negc[m]); x-rstd is
    applied only to V (head-LN on Q/K is scale-invariant, so their
    x-rstd cancels)
  - all transposes (kT, qT, avT) are DMA xbar transposes issued from
    the otherwise-idle SP engine: no PE transposes, no PSUM staging
  - scores computed transposed (scoresT[j,i]) two heads per PSUM pair
    [128,1024]; one exp per pair on ACT (phase B ACT does only exp)
  - attn@V uses attn tiles as the stationary operand and V (+ones col
    for the softmax denominator) as the 65-column moving operand:
    cost-model matmul time scales with moving columns only
  - AV lands natural [i, dh]; denominator normalize is a per-partition
    scalar multiply, then a DMA transpose produces avT for the
    out-projection
  - head-LN apply runs on GPSIMD; bn stats on DVE read PSUM directly
  - out partials are written in bf16
"""

import numpy as np

S = 2048          # sequence length
D = 2048          # model dim
H_LOC = 8         # heads per core
DH = 64           # head dim
M_LOC = H_LOC * DH  # 512 inner dim per core
N_D = D // 128    # 16 d-tiles
N_S = S // 128    # 16 s-tiles
N_SC = S // 512   # 4 512-chunks
N_MT = M_LOC // 128  # 4 m-tiles per core
EPS = 1e-5

_COMPILED = {}


def _build():
    from concourse._compat import axon_active
    axon_active()
    import concourse.bacc as bacc
    import concourse.mybir as mybir
    import concourse.tile as tile
    from contextlib import ExitStack

    F32 = mybir.dt.float32
    BF16 = mybir.dt.bfloat16
    AF = mybir.ActivationFunctionType
    OP = mybir.AluOpType

    nc = bacc.Bacc(None, target_bir_lowering=False)

    x_nat = nc.dram_tensor("x_nat", [S, D], BF16, kind="ExternalInput")
    x_tr = nc.dram_tensor("x_tr", [D, S], BF16, kind="ExternalInput")
    wgq = nc.dram_tensor("wgq", [D, M_LOC], BF16, kind="ExternalInput")
    wgk = nc.dram_tensor("wgk", [D, M_LOC], BF16, kind="ExternalInput")
    wgv = nc.dram_tensor("wgv", [D, M_LOC], BF16, kind="ExternalInput")
    negc = nc.dram_tensor("negc", [3, M_LOC], BF16, kind="ExternalInput")
    wo = nc.dram_tensor("wo", [M_LOC, D], BF16, kind="ExternalInput")
    qn_w = nc.dram_tensor("qn_w", [DH], F32, kind="ExternalInput")
    kn_w = nc.dram_tensor("kn_w", [DH], F32, kind="ExternalInput")
    out = nc.dram_tensor("out", [S, D], BF16, kind="ExternalOutput")

    import os
    DBG = bool(os.environ.get("ATTN_DEBUG"))
    dbg = {}
    if DBG:
        dbg["kT"] = nc.dram_tensor("dbg_kT", [128, N_MT, S], BF16,
                                   kind="ExternalOutput")
        dbg["vext0"] = nc.dram_tensor("dbg_vext0", [128, H_LOC, DH + 1], BF16,
                                      kind="ExternalOutput")
        dbg["qT0"] = nc.dram_tensor("dbg_qT0", [128, N_MT, 512], BF16,
                                    kind="ExternalOutput")
        dbg["avT0"] = nc.dram_tensor("dbg_avT0", [128, N_MT, 512], BF16,
                                     kind="ExternalOutput")
        dbg["attn00"] = nc.dram_tensor("dbg_attn00", [128, 1024], BF16,
                                       kind="ExternalOutput")

    with tile.TileContext(nc) as tc:
        es = ExitStack()
        # ---- pools alive for the whole kernel ----
        consts = es.enter_context(tc.tile_pool(name="consts", bufs=1))
        dram = es.enter_context(tc.tile_pool(name="dram", bufs=1, space="DRAM"))
        xt_pool = es.enter_context(tc.tile_pool(name="xt", bufs=1))
        wgq_pool = es.enter_context(tc.tile_pool(name="wgq", bufs=1))
        kT_pool = es.enter_context(tc.tile_pool(name="kT", bufs=1))
        vext_pool = es.enter_context(tc.tile_pool(name="vext", bufs=1))

        eps_t = consts.tile([128, 1], F32, name="eps_t")
        nc.vector.memset(eps_t, EPS)

        # qn/kn replicated across partitions (f32 dma, then bf16 copy)
        from concourse.bass import AP
        rep_f32 = {}
        rep_bf = {}
        for nm, wten in (("q", qn_w), ("k", kn_w)):
            rf = consts.tile([128, H_LOC, DH], F32, name=f"{nm}n_repf")
            bsrc = AP(tensor=wten[:].tensor, offset=wten[:].offset,
                      ap=[[0, 128], [0, H_LOC], [1, DH]])
            nc.scalar.dma_start(out=rf, in_=bsrc)
            rb = consts.tile([128, H_LOC * DH], BF16, name=f"{nm}n_rep")
            nc.vector.tensor_copy(rb, rf.rearrange("p h d -> p (h d)"))
            rep_f32[nm] = rf
            rep_bf[nm] = rb

        negc_sb = [consts.tile([1, M_LOC], BF16, name=f"negc_sb{r}")
                   for r in range(3)]
        for r in range(3):
            nc.scalar.dma_start(out=negc_sb[r], in_=negc[r:r + 1, :])

        # per-s-tile stat tiles
        mu_col = [consts.tile([128, 1], F32, name=f"mu_col{t}") for t in range(N_S)]
        rstd_col = [consts.tile([128, 1], F32, name=f"rstd_col{t}")
                    for t in range(N_S)]
        mu_bf = [consts.tile([1, 128], BF16, name=f"mu_bf{t}") for t in range(N_S)]
        dscr = dram.tile([S], F32)

        xt = [xt_pool.tile([128, S], BF16, name=f"xt{t}") for t in range(N_D)]
        wg_q = [wgq_pool.tile([128, M_LOC], BF16, name=f"wg_q{t}")
                for t in range(N_D)]
        kT = kT_pool.tile([128, N_MT, S], BF16, name="kT")
        v_ext = [vext_pool.tile([128, H_LOC, DH + 1], BF16, name=f"vext{st}")
                 for st in range(N_S)]

        # ============ phase A: loads + stats + K,V projections ============
        with ExitStack() as ph:
            wg_pool = ph.enter_context(tc.tile_pool(name="wg", bufs=1))
            stage = ph.enter_context(tc.tile_pool(name="stage", bufs=3))
            scrA = ph.enter_context(tc.tile_pool(name="scrA", bufs=3))
            ps_mm = ph.enter_context(tc.tile_pool(name="ps_mm", bufs=4, space="PSUM"))

            wg = {"q": wg_q}
            for wname in ("k", "v"):
                wg[wname] = [wg_pool.tile([128, M_LOC], BF16, name=f"wg_{wname}{t}")
                             for t in range(N_D)]
            wdrams = {"q": wgq, "k": wgk, "v": wgv}
            NEGC_ROW = {"q": 0, "k": 1, "v": 2}

            def emit_stats(st):
                # token mean/var via ACT accumulators (sum and sum-of-squares
                # along the free dim); DVE only does tiny [128,1] cleanups
                xst = stage.tile([128, S], BF16, tag="xst")
                nc.sync.dma_start(out=xst, in_=x_nat[st * 128:(st + 1) * 128, :])
                scr = stage.tile([128, S], BF16, tag="scr")
                s1 = scrA.tile([128, 1], F32, tag="s1")
                s2 = scrA.tile([128, 1], F32, tag="s2")
                nc.scalar.activation(out=scr, in_=xst, func=AF.Copy,
                                     accum_out=s1)
                nc.scalar.activation(out=scr, in_=xst, func=AF.Square,
                                     accum_out=s2)
                nc.vector.tensor_scalar_mul(out=mu_col[st], in0=s1,
                                            scalar1=1.0 / D)
                var = scrA.tile([128, 1], F32, tag="var")
                nc.vector.tensor_scalar_mul(out=var, in0=s2, scalar1=1.0 / D)
                mu2 = scrA.tile([128, 1], F32, tag="mu2")
                nc.vector.tensor_mul(out=mu2, in0=mu_col[st], in1=mu_col[st])
                nc.vector.tensor_sub(out=var, in0=var, in1=mu2)
                # rstd = exp(-0.5*ln(var+eps)): Ln/Exp share one ACT table
                # with the softmax Exp, so no table reloads
                nc.scalar.activation(out=rstd_col[st], in_=var,
                                     func=AF.Ln, bias=eps_t, scale=1.0)
                nc.scalar.activation(out=rstd_col[st], in_=rstd_col[st],
                                     func=AF.Exp, scale=-0.5)
                nc.scalar.dma_start(
                    out=dscr[st * 128:(st + 1) * 128].rearrange(
                        "(p one) -> p one", one=1),
                    in_=mu_col[st])
                mur = scrA.tile([1, 128], F32, tag="mur")
                nc.scalar.dma_start(
                    out=mur,
                    in_=dscr[st * 128:(st + 1) * 128].rearrange(
                        "(one s) -> one s", one=1))
                nc.vector.tensor_copy(mu_bf[st], mur)

            # input DMAs: x on SP, weights on ACT (idle in phase A)
            for t in range(N_D):
                nc.sync.dma_start(out=wg["k"][t],
                                  in_=wgk[t * 128:(t + 1) * 128, :])
                nc.sync.dma_start(out=xt[t], in_=x_tr[t * 128:(t + 1) * 128, :])
                nc.sync.dma_start(out=wg["v"][t],
                                  in_=wgv[t * 128:(t + 1) * 128, :])
                if t == 0:
                    emit_stats(0)
            for st in range(1, N_S):
                emit_stats(st)
            # Q weights: land while K/V projections run
            for t in range(N_D):
                nc.scalar.dma_start(out=wg_q[t], in_=wgq[t * 128:(t + 1) * 128, :])

            def emit_headln_tail(p, dst_T, sl, nm):
                """Head-LN on PSUM proj result p, then DMA-transpose into
                dst_T[:, :, sl]. nm selects qn/kn."""
                pg = p.rearrange("p (h d) -> p h d", d=DH)
                bn8 = scrA.tile([128, H_LOC, 6], F32, tag="bn8")
                mv8 = scrA.tile([128, H_LOC, 2], F32, tag="mv8")
                for h in range(H_LOC):
                    nc.vector.bn_stats(out=bn8[:, h, :], in_=pg[:, h, :])
                    nc.vector.bn_aggr(out=mv8[:, h, :], in_=bn8[:, h, :])
                rstd8 = scrA.tile([128, H_LOC], F32, tag="rstd8")
                nc.scalar.activation(out=rstd8, in_=mv8[:, :, 1], func=AF.Ln,
                                     bias=eps_t, scale=1.0)
                nc.scalar.activation(out=rstd8, in_=rstd8, func=AF.Exp,
                                     scale=-0.5)
                lnb = scrA.tile([128, H_LOC, DH], BF16, tag="lnb")
                for h in range(H_LOC):
                    nc.vector.tensor_scalar(
                        out=lnb[:, h, :], in0=pg[:, h, :],
                        scalar1=mv8[:, h, 0:1], scalar2=rstd8[:, h:h + 1],
                        op0=OP.subtract, op1=OP.mult)
                lnb2 = scrA.tile([128, H_LOC * DH], BF16, tag="lnb2")
                nc.gpsimd.tensor_mul(out=lnb2,
                                     in0=lnb.rearrange("p h d -> p (h d)"),
                                     in1=rep_bf[nm])
                nc.sync.dma_start_transpose(out=dst_T[:, :, sl], in_=lnb2)

            for st in range(N_S):
                sl = slice(st * 128, (st + 1) * 128)
                # K projection + head-LN + transpose
                p = ps_mm.tile([128, M_LOC], F32, tag="mm", name=f"pk{st}")
                for t in range(N_D):
                    nc.tensor.matmul(p[:, :], xt[t][:, sl], wg["k"][t][:, :],
                                     start=(t == 0), stop=False)
                nc.tensor.matmul(p[:, :], mu_bf[st][:, :],
                                 negc_sb[NEGC_ROW["k"]][:, :],
                                 start=False, stop=True)
                emit_headln_tail(p, kT, sl, "k")
                # V projection + x-rstd + ones column
                p2 = ps_mm.tile([128, M_LOC], F32, tag="mm", name=f"pv{st}")
                for t in range(N_D):
                    nc.tensor.matmul(p2[:, :], xt[t][:, sl], wg["v"][t][:, :],
                                     start=(t == 0), stop=False)
                nc.tensor.matmul(p2[:, :], mu_bf[st][:, :],
                                 negc_sb[NEGC_ROW["v"]][:, :],
                                 start=False, stop=True)
                nc.vector.tensor_scalar_mul(
                    out=v_ext[st][:, :, 0:DH],
                    in0=p2.rearrange("p (h d) -> p h d", d=DH),
                    scalar1=rstd_col[st])
                nc.vector.memset(v_ext[st][:, :, DH:DH + 1], 1.0)

        if DBG:
            nc.sync.dma_start(out=dbg["kT"][:, :, :], in_=kT)
            nc.sync.dma_start(out=dbg["vext0"][:, :, :], in_=v_ext[0])

        # ============ phase B: Q proj + attention + out-proj ============
        with ExitStack() as ph:
            wo_pool = ph.enter_context(tc.tile_pool(name="wop", bufs=1))
            qT_pool = ph.enter_context(tc.tile_pool(name="qT", bufs=2))
            attn_pool = ph.enter_context(tc.tile_pool(name="attn", bufs=12))
            avsb_pool = ph.enter_context(tc.tile_pool(name="avsb", bufs=2))
            avT_pool = ph.enter_context(tc.tile_pool(name="avT", bufs=2))
            ot_pool = ph.enter_context(tc.tile_pool(name="ot", bufs=3))
            scrB = ph.enter_context(tc.tile_pool(name="scrB", bufs=3))
            # PSUM: sc 2x2 banks + av 2x1 + qp 1 + op 1 = 8 banks
            ps_sc = ph.enter_context(tc.tile_pool(name="ps_sc", bufs=2,
                                                  space="PSUM"))
            ps_av = ph.enter_context(tc.tile_pool(name="ps_av", bufs=2,
                                                  space="PSUM"))
            ps_qp = ph.enter_context(tc.tile_pool(name="ps_qp", bufs=1,
                                                  space="PSUM"))
            ps_op = ph.enter_context(tc.tile_pool(name="ps_op", bufs=1,
                                                  space="PSUM"))

            wo_sb = [wo_pool.tile([128, D], BF16, name=f"wo{mt}")
                     for mt in range(N_MT)]
            for mt in range(N_MT):
                nc.scalar.dma_start(out=wo_sb[mt],
                                    in_=wo[mt * 128:(mt + 1) * 128, :])

            def emit_qtail(st, p, qT_next):
                ssl = slice((st % 4) * 128, (st % 4 + 1) * 128)
                nc.tensor.matmul(p[:, :], mu_bf[st][:, :],
                                 negc_sb[NEGC_ROW["q"]][:, :],
                                 start=False, stop=True)
                pg = p.rearrange("p (h d) -> p h d", d=DH)
                bn8 = scrB.tile([128, H_LOC, 6], F32, tag="bn8")
                mv8 = scrB.tile([128, H_LOC, 2], F32, tag="mv8")
                for h in range(H_LOC):
                    nc.vector.bn_stats(out=bn8[:, h, :], in_=pg[:, h, :])
                    nc.vector.bn_aggr(out=mv8[:, h, :], in_=bn8[:, h, :])
                rstd8 = scrB.tile([128, H_LOC], F32, tag="rstd8")
                nc.scalar.activation(out=rstd8, in_=mv8[:, :, 1], func=AF.Ln,
                                     bias=eps_t, scale=1.0)
                nc.scalar.activation(out=rstd8, in_=rstd8, func=AF.Exp,
                                     scale=-0.5)
                lnb = scrB.tile([128, H_LOC, DH], BF16, tag="lnb")
                for h in range(H_LOC):
                    nc.vector.tensor_scalar(
                        out=lnb[:, h, :], in0=pg[:, h, :],
                        scalar1=mv8[:, h, 0:1], scalar2=rstd8[:, h:h + 1],
                        op0=OP.subtract, op1=OP.mult)
                lnb2 = scrB.tile([128, H_LOC * DH], BF16, tag="lnb2")
                nc.vector.tensor_mul(out=lnb2,
                                     in0=lnb.rearrange("p h d -> p (h d)"),
                                     in1=rep_bf["q"])
                nc.sync.dma_start_transpose(out=qT_next[:, :, ssl], in_=lnb2)

            def emit_oproj(avT_src, st, do):
                sl = slice(st * 128, (st + 1) * 128)
                lsl = slice((st % 4) * 128, (st % 4 + 1) * 128)
                op = ps_op.tile([128, 512], F32, tag="op", name=f"op{st}_{do}")
                for mt in range(N_MT):
                    nc.tensor.matmul(op[:, :], avT_src[:, mt, lsl],
                                     wo_sb[mt][:, do * 512:(do + 1) * 512],
                                     start=(mt == 0), stop=(mt == N_MT - 1))
                ot = ot_pool.tile([128, 512], BF16, tag="ot")
                nc.vector.tensor_copy(ot, op)
                nc.gpsimd.dma_start(out=out[sl, do * 512:(do + 1) * 512],
                                    in_=ot)

            def alloc_qT(ic):
                return qT_pool.tile([128, N_MT, 512], BF16, tag="qT",
                                    name=f"qT{ic}")

            qT_cur = alloc_qT(0)
            for st in range(4):
                qp = ps_qp.tile([128, M_LOC], F32, tag="qp", name=f"qp{st}")
                qsl = slice(st * 128, (st + 1) * 128)
                for t in range(N_D):
                    nc.tensor.matmul(qp[:, :], xt[t][:, qsl], wg_q[t][:, :],
                                     start=(t == 0), stop=False)
                emit_qtail(st, qp, qT_cur)

            avT_prev = None
            for ic in range(N_SC):
                qT_next = alloc_qT(ic + 1) if ic + 1 < N_SC else None
                avT_cur = avT_pool.tile([128, N_MT, 512], BF16, tag="avT",
                                        name=f"avT{ic}")
                for hp in range(N_MT):
                    qst = 4 * (ic + 1) + hp if qT_next is not None else None
                    qp = None
                    if qst is not None:
                        qp = ps_qp.tile([128, M_LOC], F32, tag="qp",
                                        name=f"qp{qst}")
                    avsb = avsb_pool.tile([128, 4, 2, DH], BF16, tag="avsb",
                                          name=f"avsb{ic}_{hp}")

                    def emit_av_chain(attn_tiles, hs, it, h):
                        # one 16-step accumulation chain in its own PSUM bank
                        # (in-flight chains sharing a bank corrupt each other),
                        # then denominator normalize into avsb
                        av = ps_av.tile([128, DH + 1], F32, tag="av",
                                        name=f"av{ic}_{hp}_{hs}_{it}")
                        for pj in range(8):
                            for half in range(2):
                                csl = slice(half * 512 + it * 128,
                                            half * 512 + (it + 1) * 128)
                                nc.tensor.matmul(
                                    av[:, 0:DH + 1], attn_tiles[pj][:, csl],
                                    v_ext[2 * pj + half][:, h, :],
                                    start=(pj == 0 and half == 0),
                                    stop=(pj == 7 and half == 1))
                        nc.vector.reciprocal(av[:, DH:DH + 1], av[:, DH:DH + 1])
                        nc.vector.tensor_scalar_mul(
                            out=avsb[:, it, hs, :], in0=av[:, 0:DH],
                            scalar1=av[:, DH:DH + 1])

                    qp_step = [0]

                    def emit_qp_step():
                        if qp is not None and qp_step[0] < N_D:
                            t = qp_step[0]
                            qsl = slice(qst * 128, (qst + 1) * 128)
                            nc.tensor.matmul(qp[:, :], xt[t][:, qsl],
                                             wg_q[t][:, :],
                                             start=(t == 0), stop=False)
                            qp_step[0] += 1

                    for hs in range(2):
                        h = 2 * hp + hs
                        psl = slice(hs * DH, (hs + 1) * DH)
                        attn_tiles = []
                        for pj in range(8):
                            if avT_prev is not None and pj % 4 == 1:
                                emit_oproj(avT_prev, 4 * (ic - 1) + hp,
                                           2 * hs + pj // 4)
                            sc = ps_sc.tile([128, 1024], F32, tag="sc",
                                            name=f"sc{hs}_{pj}")
                            for half in range(2):
                                jt = 2 * pj + half
                                jsl = slice(jt * 128, (jt + 1) * 128)
                                nc.tensor.matmul(
                                    sc[:, half * 512:(half + 1) * 512],
                                    kT[psl, hp, jsl], qT_cur[psl, hp, :],
                                    start=True, stop=True)
                            a = attn_pool.tile([128, 1024], BF16, tag="attn")
                            nc.scalar.activation(out=a, in_=sc, func=AF.Exp,
                                                 scale=0.125)
                            if DBG and ic == 0 and hp == 0 and hs == 0 \
                                    and pj == 0:
                                nc.sync.dma_start(out=dbg["attn00"][:, :],
                                                  in_=a)
                            attn_tiles.append(a)
                            emit_qp_step()
                        # pass 1: chains for i-blocks 0,1 (banks A,B)
                        for it in range(2):
                            emit_av_chain(attn_tiles, hs, it, h)
                        # pass 2: chains for i-blocks 2,3 reuse the banks
                        for it in range(2, 4):
                            emit_av_chain(attn_tiles, hs, it, h)
                    nc.sync.dma_start_transpose(
                        out=avT_cur[:, hp, :].rearrange(
                            "p (t f) -> p t f", f=128),
                        in_=avsb.rearrange("p t h d -> p (t h d)"))
                    if qp is not None:
                        emit_qtail(qst, qp, qT_next)
                if DBG and ic == 0:
                    nc.sync.dma_start(out=dbg["qT0"][:, :, :], in_=qT_cur)
                    nc.sync.dma_start(out=dbg["avT0"][:, :, :], in_=avT_cur)
                avT_prev = avT_cur
                qT_cur = qT_next

            # tail: out-projection of the last chunk
            for st in range(4 * (N_SC - 1), 4 * N_SC):
                for do in range(N_SC):
                    emit_oproj(avT_prev, st, do)
        es.close()

    # The act-table-load inserter picks the first table containing each
    # activation's function; Exp appears in three tables while Ln is only
    # in natural_log_exp_and_others, which makes Exp<->Ln alternation
    # reload tables constantly. Prune Exp from the other tables (set ids
    # keep their positions) so every function resolves to the shared one
    # and the table is loaded exactly once.
    import concourse.bacc as bacc_mod
    orig_tables = bacc_mod.get_activation_tables

    def pruned_tables(arch):
        tabs = orig_tables(arch)
        return {
            name: (s if name == "natural_log_exp_and_others"
                   else s - {AF.Exp, AF.Square, AF.Copy, AF.Identity})
            for name, s in tabs.items()
        }

    bacc_mod.get_activation_tables = pruned_tables
    try:
        nc.compile()
    finally:
        bacc_mod.get_activation_tables = orig_tables
    return nc


def _get_nc():
    if "nc" not in _COMPILED:
        _COMPILED["nc"] = _build()
    return _COMPILED["nc"]


def kernel(x, norm_w, wq, wk, wv, qn_w, kn_w, wo):
    import ml_dtypes
    from concourse.bass_utils import run_bass_kernel_spmd

    x = np.asarray(x, dtype=np.float32)
    norm_w = np.asarray(norm_w, dtype=np.float32)
    wq = np.asarray(wq, dtype=np.float32)
    wk = np.asarray(wk, dtype=np.float32)
    wv = np.asarray(wv, dtype=np.float32)
    qn_w = np.asarray(qn_w, dtype=np.float32)
    kn_w = np.asarray(kn_w, dtype=np.float32)
    wo = np.asarray(wo, dtype=np.float32)
    B = x.shape[0]

    nc = _get_nc()
    in_maps = []
    for c in range(8):
        b, g = c // 4, c % 4
        ms = slice(g * M_LOC, (g + 1) * M_LOC)
        gq = norm_w[:, None] * wq[:, ms]
        gk = norm_w[:, None] * wk[:, ms]
        gv = norm_w[:, None] * wv[:, ms]
        negc = -np.stack([gq.sum(0), gk.sum(0), gv.sum(0)])
        in_maps.append({
            "x_nat": np.ascontiguousarray(x[b]).astype(ml_dtypes.bfloat16),
            "x_tr": np.ascontiguousarray(x[b].T).astype(ml_dtypes.bfloat16),
            "wgq": np.ascontiguousarray(gq).astype(ml_dtypes.bfloat16),
            "wgk": np.ascontiguousarray(gk).astype(ml_dtypes.bfloat16),
            "wgv": np.ascontiguousarray(gv).astype(ml_dtypes.bfloat16),
            "negc": np.ascontiguousarray(negc).astype(ml_dtypes.bfloat16),
            "wo": np.ascontiguousarray(wo[ms, :]).astype(ml_dtypes.bfloat16),
            "qn_w": qn_w,
            "kn_w": kn_w,
        })
    res = run_bass_kernel_spmd(nc, in_maps, core_ids=list(range(8)))
    out = np.zeros((B, S, D), dtype=np.float32)
    for c in range(8):
        out[c // 4] += np.asarray(res.results[c]["out"], dtype=np.float32)
    return out


# revision 13
# speedup vs baseline: 1.1046x; 1.0756x over previous
"""Trainium2 Bass kernel for nn_Attention_55894704390617.

Dense transformer attention block:
  xn = LN(x) ; q,k,v = xn @ wq/wk/wv ; q,k = headLN(q),headLN(k)
  out = softmax(q k^T / sqrt(dh)) v @ wo

Sharding over 8 NeuronCores: 2 (batch) x 4 (head groups of 8 heads).
Each core computes a partial output (its head-group's contribution to
out = attn_out @ wo); the host sums the 4 partials per batch.

Per-core data flow (matmuls in bf16, fp32 PSUM accumulation):
  - host pre-folds norm_w into wq/wk/wv and ships the transposed x and
    the -colsum(w) correction rows; LN mean is folded into each
    projection as a K=1 accumulation row (mu[s] x negc[m]); x-rstd is
    applied only to V (head-LN on Q/K is scale-invariant, so their
    x-rstd cancels)
  - all transposes (kT, qT, avT) are DMA xbar transposes issued from
    the otherwise-idle SP engine: no PE transposes, no PSUM staging
  - scores computed transposed (scoresT[j,i]) two heads per PSUM pair
    [128,1024]; one exp per pair on ACT (phase B ACT does only exp)
  - attn@V uses attn tiles as the stationary operand and V (+ones col
    for the softmax denominator) as the 65-column moving operand:
    cost-model matmul time scales with moving columns only
  - AV lands natural [i, dh]; denominator normalize is a per-partition
    scalar multiply, then a DMA transpose produces avT for the
    out-projection
  - head-LN apply runs on GPSIMD; bn stats on DVE read PSUM directly
  - out partials are written in bf16
"""

import numpy as np

S = 2048          # sequence length
D = 2048          # model dim
H_LOC = 8         # heads per core
DH = 64           # head dim
M_LOC = H_LOC * DH  # 512 inner dim per core
N_D = D // 128    # 16 d-tiles
N_S = S // 128    # 16 s-tiles
N_SC = S // 512   # 4 512-chunks
N_MT = M_LOC // 128  # 4 m-tiles per core
EPS = 1e-5

_COMPILED = {}


def _build():
    from concourse._compat import axon_active
    axon_active()
    import concourse.bacc as bacc
    import concourse.mybir as mybir
    import concourse.tile as tile
    from contextlib import ExitStack

    F32 = mybir.dt.float32
    BF16 = mybir.dt.bfloat16
    AF = mybir.ActivationFunctionType
    OP = mybir.AluOpType

    nc = bacc.Bacc(None, target_bir_lowering=False)

    x_nat = nc.dram_tensor("x_nat", [S, D], BF16, kind="ExternalInput")
    x_tr = nc.dram_tensor("x_tr", [D, S], BF16, kind="ExternalInput")
    wgq = nc.dram_tensor("wgq", [D, M_LOC], BF16, kind="ExternalInput")
    wgk = nc.dram_tensor("wgk", [D, M_LOC], BF16, kind="ExternalInput")
    wgv = nc.dram_tensor("wgv", [D, M_LOC], BF16, kind="ExternalInput")
    negc = nc.dram_tensor("negc", [3, M_LOC], BF16, kind="ExternalInput")
    wo = nc.dram_tensor("wo", [M_LOC, D], BF16, kind="ExternalInput")
    qn_w = nc.dram_tensor("qn_w", [DH], F32, kind="ExternalInput")
    kn_w = nc.dram_tensor("kn_w", [DH], F32, kind="ExternalInput")
    out = nc.dram_tensor("out", [S, D], BF16, kind="ExternalOutput")

    import os
    DBG = bool(os.environ.get("ATTN_DEBUG"))
    dbg = {}
    if DBG:
        dbg["kT"] = nc.dram_tensor("dbg_kT", [128, N_MT, S], BF16,
                                   kind="ExternalOutput")
        dbg["vext0"] = nc.dram_tensor("dbg_vext0", [128, H_LOC, DH + 1], BF16,
                                      kind="ExternalOutput")
        dbg["qT0"] = nc.dram_tensor("dbg_qT0", [128, N_MT, 512], BF16,
                                    kind="ExternalOutput")
        dbg["avT0"] = nc.dram_tensor("dbg_avT0", [128, N_MT, 512], BF16,
                                     kind="ExternalOutput")
        dbg["attn00"] = nc.dram_tensor("dbg_attn00", [128, 1024], BF16,
                                       kind="ExternalOutput")

    with tile.TileContext(nc) as tc:
        es = ExitStack()
        # ---- pools alive for the whole kernel ----
        consts = es.enter_context(tc.tile_pool(name="consts", bufs=1))
        dram = es.enter_context(tc.tile_pool(name="dram", bufs=1, space="DRAM"))
        xt_pool = es.enter_context(tc.tile_pool(name="xt", bufs=1))
        wgq_pool = es.enter_context(tc.tile_pool(name="wgq", bufs=1))
        kT_pool = es.enter_context(tc.tile_pool(name="kT", bufs=1))
        vext_pool = es.enter_context(tc.tile_pool(name="vext", bufs=1))

        eps_t = consts.tile([128, 1], F32, name="eps_t")
        nc.vector.memset(eps_t, EPS)
        from concourse.masks import make_identity
        ident = consts.tile([128, 128], BF16, name="ident")
        make_identity(nc, ident)

        # qn/kn replicated across partitions (f32 dma, then bf16 copy)
        from concourse.bass import AP
        rep_f32 = {}
        rep_bf = {}
        for nm, wten in (("q", qn_w), ("k", kn_w)):
            rf = consts.tile([128, H_LOC, DH], F32, name=f"{nm}n_repf")
            bsrc = AP(tensor=wten[:].tensor, offset=wten[:].offset,
                      ap=[[0, 128], [0, H_LOC], [1, DH]])
            nc.scalar.dma_start(out=rf, in_=bsrc)
            rb = consts.tile([128, H_LOC * DH], BF16, name=f"{nm}n_rep")
            nc.vector.tensor_copy(rb, rf.rearrange("p h d -> p (h d)"))
            rep_f32[nm] = rf
            rep_bf[nm] = rb

        negc_sb = [consts.tile([1, M_LOC], BF16, name=f"negc_sb{r}")
                   for r in range(3)]
        for r in range(3):
            nc.scalar.dma_start(out=negc_sb[r], in_=negc[r:r + 1, :])

        # per-s-tile stat tiles
        mu_col = [consts.tile([128, 1], F32, name=f"mu_col{t}") for t in range(N_S)]
        rstd_col = [consts.tile([128, 1], F32, name=f"rstd_col{t}")
                    for t in range(N_S)]
        mu_bf = [consts.tile([1, 128], BF16, name=f"mu_bf{t}") for t in range(N_S)]

        xt = [xt_pool.tile([128, S], BF16, name=f"xt{t}") for t in range(N_D)]
        wg_q = [wgq_pool.tile([128, M_LOC], BF16, name=f"wg_q{t}")
                for t in range(N_D)]
        kT = kT_pool.tile([128, N_MT, S], BF16, name="kT")
        v_ext = [vext_pool.tile([128, H_LOC, DH + 1], BF16, name=f"vext{st}")
                 for st in range(N_S)]

        # ============ phase A: loads + stats + K,V projections ============
        with ExitStack() as ph:
            wg_pool = ph.enter_context(tc.tile_pool(name="wg", bufs=1))
            stage = ph.enter_context(tc.tile_pool(name="stage", bufs=3))
            scrA = ph.enter_context(tc.tile_pool(name="scrA", bufs=3))
            ps_mm = ph.enter_context(tc.tile_pool(name="ps_mm", bufs=4, space="PSUM"))
            ps_mu = ph.enter_context(tc.tile_pool(name="ps_mu", bufs=2,
                                                  space="PSUM"))

            wg = {"q": wg_q}
            for wname in ("k", "v"):
                wg[wname] = [wg_pool.tile([128, M_LOC], BF16, name=f"wg_{wname}{t}")
                             for t in range(N_D)]
            wdrams = {"q": wgq, "k": wgk, "v": wgv}
            NEGC_ROW = {"q": 0, "k": 1, "v": 2}

            def emit_stats(st):
                # token mean/var via ACT accumulators (sum and sum-of-squares
                # along the free dim); DVE only does tiny [128,1] cleanups
                xst = stage.tile([128, S], BF16, tag="xst")
                nc.sync.dma_start(out=xst, in_=x_nat[st * 128:(st + 1) * 128, :])
                scr = stage.tile([128, S], BF16, tag="scr")
                s1 = scrA.tile([128, 1], F32, tag="s1")
                s2 = scrA.tile([128, 1], F32, tag="s2")
                nc.scalar.activation(out=scr, in_=xst, func=AF.Copy,
                                     accum_out=s1)
                nc.scalar.activation(out=scr, in_=xst, func=AF.Square,
                                     accum_out=s2)
                nc.vector.tensor_scalar_mul(out=mu_col[st], in0=s1,
                                            scalar1=1.0 / D)
                mu_cb = scrA.tile([128, 1], BF16, tag="mu_cb")
                nc.vector.tensor_copy(mu_cb, mu_col[st])
                mu_ps = ps_mu.tile([1, 128], F32, tag="mu",
                                   name=f"mu_ps{st}")
                nc.tensor.matmul(mu_ps[:, :], mu_cb[:, :], ident[:, :],
                                 start=True, stop=True)
                nc.vector.tensor_copy(mu_bf[st], mu_ps)
                var = scrA.tile([128, 1], F32, tag="var")
                nc.vector.tensor_scalar_mul(out=var, in0=s2, scalar1=1.0 / D)
                mu2 = scrA.tile([128, 1], F32, tag="mu2")
                nc.vector.tensor_mul(out=mu2, in0=mu_col[st], in1=mu_col[st])
                nc.vector.tensor_sub(out=var, in0=var, in1=mu2)
                # rstd = exp(-0.5*ln(var+eps)): Ln/Exp share one ACT table
                # with the softmax Exp, so no table reloads
                nc.scalar.activation(out=rstd_col[st], in_=var,
                                     func=AF.Ln, bias=eps_t, scale=1.0)
                nc.scalar.activation(out=rstd_col[st], in_=rstd_col[st],
                                     func=AF.Exp, scale=-0.5)
                nc.vector.tensor_scalar_mul(out=mu_col[st], in0=s1,
                                            scalar1=1.0 / D)
                mu_cb = scrA.tile([128, 1], BF16, tag="mu_cb")
                nc.vector.tensor_copy(mu_cb, mu_col[st])
                mu_ps = ps_mu.tile([1, 128], F32, tag="mu",
                                   name=f"mu_ps{st}")
                nc.tensor.matmul(mu_ps[:, :], mu_cb[:, :], ident[:, :],
                                 start=True, stop=True)
                nc.vector.tensor_copy(mu_bf[st], mu_ps)
            # input DMAs: x on SP, weights on ACT (idle in phase A)
            for t in range(N_D):
                nc.sync.dma_start(out=wg["k"][t],
                                  in_=wgk[t * 128:(t + 1) * 128, :])
                nc.sync.dma_start(out=xt[t], in_=x_tr[t * 128:(t + 1) * 128, :])
                nc.sync.dma_start(out=wg["v"][t],
                                  in_=wgv[t * 128:(t + 1) * 128, :])
                emit_stats(t)
            # Q weights: land while K/V projections run
            for t in range(N_D):
                nc.scalar.dma_start(out=wg_q[t], in_=wgq[t * 128:(t + 1) * 128, :])

            def emit_headln_tail(p, dst_T, sl, nm):
                """Head-LN on PSUM proj result p, then DMA-transpose into
                dst_T[:, :, sl]. nm selects qn/kn."""
                pg = p.rearrange("p (h d) -> p h d", d=DH)
                bn8 = scrA.tile([128, H_LOC, 6], F32, tag="bn8")
                mv8 = scrA.tile([128, H_LOC, 2], F32, tag="mv8")
                for h in range(H_LOC):
                    nc.vector.bn_stats(out=bn8[:, h, :], in_=pg[:, h, :])
                    nc.vector.bn_aggr(out=mv8[:, h, :], in_=bn8[:, h, :])
                rstd8 = scrA.tile([128, H_LOC], F32, tag="rstd8")
                nc.scalar.activation(out=rstd8, in_=mv8[:, :, 1], func=AF.Ln,
                                     bias=eps_t, scale=1.0)
                nc.scalar.activation(out=rstd8, in_=rstd8, func=AF.Exp,
                                     scale=-0.5)
                lnb = scrA.tile([128, H_LOC, DH], BF16, tag="lnb")
                for h in range(H_LOC):
                    nc.vector.tensor_scalar(
                        out=lnb[:, h, :], in0=pg[:, h, :],
                        scalar1=mv8[:, h, 0:1], scalar2=rstd8[:, h:h + 1],
                        op0=OP.subtract, op1=OP.mult)
                lnb2 = scrA.tile([128, H_LOC * DH], BF16, tag="lnb2")
                nc.gpsimd.tensor_mul(out=lnb2,
                                     in0=lnb.rearrange("p h d -> p (h d)"),
                                     in1=rep_bf[nm])
                nc.sync.dma_start_transpose(out=dst_T[:, :, sl], in_=lnb2)

            for st in range(N_S):
                sl = slice(st * 128, (st + 1) * 128)
                # K projection + head-LN + transpose
                p = ps_mm.tile([128, M_LOC], F32, tag="mm", name=f"pk{st}")
                for t in range(N_D):
                    nc.tensor.matmul(p[:, :], xt[t][:, sl], wg["k"][t][:, :],
                                     start=(t == 0), stop=False)
                nc.tensor.matmul(p[:, :], mu_bf[st][:, :],
                                 negc_sb[NEGC_ROW["k"]][:, :],
                                 start=False, stop=True)
                emit_headln_tail(p, kT, sl, "k")
                # V projection + x-rstd + ones column
                p2 = ps_mm.tile([128, M_LOC], F32, tag="mm", name=f"pv{st}")
                for t in range(N_D):
                    nc.tensor.matmul(p2[:, :], xt[t][:, sl], wg["v"][t][:, :],
                                     start=(t == 0), stop=False)
                nc.tensor.matmul(p2[:, :], mu_bf[st][:, :],
                                 negc_sb[NEGC_ROW["v"]][:, :],
                                 start=False, stop=True)
                nc.vector.tensor_scalar_mul(
                    out=v_ext[st][:, :, 0:DH],
                    in0=p2.rearrange("p (h d) -> p h d", d=DH),
                    scalar1=rstd_col[st])
                nc.vector.memset(v_ext[st][:, :, DH:DH + 1], 1.0)

        if DBG:
            nc.sync.dma_start(out=dbg["kT"][:, :, :], in_=kT)
            nc.sync.dma_start(out=dbg["vext0"][:, :, :], in_=v_ext[0])

        # ============ phase B: Q proj + attention + out-proj ============
        with ExitStack() as ph:
            wo_pool = ph.enter_context(tc.tile_pool(name="wop", bufs=1))
            qT_pool = ph.enter_context(tc.tile_pool(name="qT", bufs=2))
            attn_pool = ph.enter_context(tc.tile_pool(name="attn", bufs=12))
            avsb_pool = ph.enter_context(tc.tile_pool(name="avsb", bufs=2))
            avT_pool = ph.enter_context(tc.tile_pool(name="avT", bufs=2))
            ot_pool = ph.enter_context(tc.tile_pool(name="ot", bufs=3))
            scrB = ph.enter_context(tc.tile_pool(name="scrB", bufs=3))
            # PSUM: sc 2x2 banks + av 2x1 + qp 1 + op 1 = 8 banks
            ps_sc = ph.enter_context(tc.tile_pool(name="ps_sc", bufs=2,
                                                  space="PSUM"))
            ps_av = ph.enter_context(tc.tile_pool(name="ps_av", bufs=2,
                                                  space="PSUM"))
            ps_qp = ph.enter_context(tc.tile_pool(name="ps_qp", bufs=1,
                                                  space="PSUM"))
            ps_op = ph.enter_context(tc.tile_pool(name="ps_op", bufs=1,
                                                  space="PSUM"))

            wo_sb = [wo_pool.tile([128, D], BF16, name=f"wo{mt}")
                     for mt in range(N_MT)]
            for mt in range(N_MT):
                nc.scalar.dma_start(out=wo_sb[mt],
                                    in_=wo[mt * 128:(mt + 1) * 128, :])

            def emit_qtail(st, p, qT_next):
                ssl = slice((st % 4) * 128, (st % 4 + 1) * 128)
                nc.tensor.matmul(p[:, :], mu_bf[st][:, :],
                                 negc_sb[NEGC_ROW["q"]][:, :],
                                 start=False, stop=True)
                pg = p.rearrange("p (h d) -> p h d", d=DH)
                bn8 = scrB.tile([128, H_LOC, 6], F32, tag="bn8")
                mv8 = scrB.tile([128, H_LOC, 2], F32, tag="mv8")
                for h in range(H_LOC):
                    nc.vector.bn_stats(out=bn8[:, h, :], in_=pg[:, h, :])
                    nc.vector.bn_aggr(out=mv8[:, h, :], in_=bn8[:, h, :])
                rstd8 = scrB.tile([128, H_LOC], F32, tag="rstd8")
                nc.scalar.activation(out=rstd8, in_=mv8[:, :, 1], func=AF.Ln,
                                     bias=eps_t, scale=1.0)
                nc.scalar.activation(out=rstd8, in_=rstd8, func=AF.Exp,
                                     scale=-0.5)
                lnb = scrB.tile([128, H_LOC, DH], BF16, tag="lnb")
                for h in range(H_LOC):
                    nc.vector.tensor_scalar(
                        out=lnb[:, h, :], in0=pg[:, h, :],
                        scalar1=mv8[:, h, 0:1], scalar2=rstd8[:, h:h + 1],
                        op0=OP.subtract, op1=OP.mult)
                lnb2 = scrB.tile([128, H_LOC * DH], BF16, tag="lnb2")
                nc.vector.tensor_mul(out=lnb2,
                                     in0=lnb.rearrange("p h d -> p (h d)"),
                                     in1=rep_bf["q"])
                nc.sync.dma_start_transpose(out=qT_next[:, :, ssl], in_=lnb2)

            def emit_oproj(avT_src, st, do):
                sl = slice(st * 128, (st + 1) * 128)
                lsl = slice((st % 4) * 128, (st % 4 + 1) * 128)
                op = ps_op.tile([128, 512], F32, tag="op", name=f"op{st}_{do}")
                for mt in range(N_MT):
                    nc.tensor.matmul(op[:, :], avT_src[:, mt, lsl],
                                     wo_sb[mt][:, do * 512:(do + 1) * 512],
                                     start=(mt == 0), stop=(mt == N_MT - 1))
                ot = ot_pool.tile([128, 512], BF16, tag="ot")
                nc.vector.tensor_copy(ot, op)
                nc.gpsimd.dma_start(out=out[sl, do * 512:(do + 1) * 512],
                                    in_=ot)

            def alloc_qT(ic):
                return qT_pool.tile([128, N_MT, 512], BF16, tag="qT",
                                    name=f"qT{ic}")

            qT_cur = alloc_qT(0)
            for st in range(4):
                qp = ps_qp.tile([128, M_LOC], F32, tag="qp", name=f"qp{st}")
                qsl = slice(st * 128, (st + 1) * 128)
                for t in range(N_D):
                    nc.tensor.matmul(qp[:, :], xt[t][:, qsl], wg_q[t][:, :],
                                     start=(t == 0), stop=False)
                emit_qtail(st, qp, qT_cur)

            avT_prev = None
            for ic in range(N_SC):
                qT_next = alloc_qT(ic + 1) if ic + 1 < N_SC else None
                avT_cur = avT_pool.tile([128, N_MT, 512], BF16, tag="avT",
                                        name=f"avT{ic}")
                for hp in range(N_MT):
                    qst = 4 * (ic + 1) + hp if qT_next is not None else None
                    qp = None
                    if qst is not None:
                        qp = ps_qp.tile([128, M_LOC], F32, tag="qp",
                                        name=f"qp{qst}")
                    avsb = avsb_pool.tile([128, 4, 2, DH], BF16, tag="avsb",
                                          name=f"avsb{ic}_{hp}")

                    def emit_av_chain(attn_tiles, hs, it, h):
                        # one 16-step accumulation chain in its own PSUM bank
                        # (in-flight chains sharing a bank corrupt each other),
                        # then denominator normalize into avsb
                        av = ps_av.tile([128, DH + 1], F32, tag="av",
                                        name=f"av{ic}_{hp}_{hs}_{it}")
                        for pj in range(8):
                            for half in range(2):
                                csl = slice(half * 512 + it * 128,
                                            half * 512 + (it + 1) * 128)
                                nc.tensor.matmul(
                                    av[:, 0:DH + 1], attn_tiles[pj][:, csl],
                                    v_ext[2 * pj + half][:, h, :],
                                    start=(pj == 0 and half == 0),
                                    stop=(pj == 7 and half == 1))
                        nc.vector.reciprocal(av[:, DH:DH + 1], av[:, DH:DH + 1])
                        nc.vector.tensor_scalar_mul(
                            out=avsb[:, it, hs, :], in0=av[:, 0:DH],
                            scalar1=av[:, DH:DH + 1])

                    qp_step = [0]

                    def emit_qp_step():
                        if qp is not None and qp_step[0] < N_D:
                            t = qp_step[0]
                            qsl = slice(qst * 128, (qst + 1) * 128)
                            nc.tensor.matmul(qp[:, :], xt[t][:, qsl],
                                             wg_q[t][:, :],
                                             start=(t == 0), stop=False)
                            qp_step[0] += 1

                    for hs in range(2):
                        h = 2 * hp + hs
                        psl = slice(hs * DH, (hs + 1) * DH)
                        attn_tiles = []
                        for pj in range(8):
                            if avT_prev is not None and pj % 4 == 1:
                                emit_oproj(avT_prev, 4 * (ic - 1) + hp,
                                           2 * hs + pj // 4)
                            sc = ps_sc.tile([128, 1024], F32, tag="sc",
                                            name=f"sc{hs}_{pj}")
                            for half in range(2):
                                jt = 2 * pj + half
                                jsl = slice(jt * 128, (jt + 1) * 128)
                                nc.tensor.matmul(
                                    sc[:, half * 512:(half + 1) * 512],
                                    kT[psl, hp, jsl], qT_cur[psl, hp, :],
                                    start=True, stop=True)
                            a = attn_pool.tile([128, 1024], BF16, tag="attn")
                            nc.scalar.activation(out=a, in_=sc, func=AF.Exp,
                                                 scale=0.125)
                            if DBG and ic == 0 and hp == 0 and hs == 0 \
                                    and pj == 0:
                                nc.sync.dma_start(out=dbg["attn00"][:, :],
                                                  in_=a)
                            attn_tiles.append(a)
                            emit_qp_step()
                        # pass 1: chains for i-blocks 0,1 (banks A,B)
                        for it in range(2):
                            emit_av_chain(attn_tiles, hs, it, h)
                        # pass 2: chains for i-blocks 2,3 reuse the banks
                        for it in range(2, 4):
                            emit_av_chain(attn_tiles, hs, it, h)
                    nc.sync.dma_start_transpose(
                        out=avT_cur[:, hp, :].rearrange(
                            "p (t f) -> p t f", f=128),
                        in_=avsb.rearrange("p t h d -> p (t h d)"))
                    if qp is not None:
                        emit_qtail(qst, qp, qT_next)
                if DBG and ic == 0:
                    nc.sync.dma_start(out=dbg["qT0"][:, :, :], in_=qT_cur)
                    nc.sync.dma_start(out=dbg["avT0"][:, :, :], in_=avT_cur)
                avT_prev = avT_cur
                qT_cur = qT_next

            # tail: out-projection of the last chunk
            for st in range(4 * (N_SC - 1), 4 * N_SC):
                for do in range(N_SC):
                    emit_oproj(avT_prev, st, do)
        es.close()

    # The act-table-load inserter picks the first table containing each
    # activation's function; Exp appears in three tables while Ln is only
    # in natural_log_exp_and_others, which makes Exp<->Ln alternation
    # reload tables constantly. Prune Exp from the other tables (set ids
    # keep their positions) so every function resolves to the shared one
    # and the table is loaded exactly once.
    import concourse.bacc as bacc_mod
    orig_tables = bacc_mod.get_activation_tables

    def pruned_tables(arch):
        tabs = orig_tables(arch)
        return {
            name: (s if name == "natural_log_exp_and_others"
                   else s - {AF.Exp, AF.Square, AF.Copy, AF.Identity})
            for name, s in tabs.items()
        }

    bacc_mod.get_activation_tables = pruned_tables
    try:
        nc.compile()
    finally:
        bacc_mod.get_activation_tables = orig_tables
    return nc


def _get_nc():
    if "nc" not in _COMPILED:
        _COMPILED["nc"] = _build()
    return _COMPILED["nc"]


def kernel(x, norm_w, wq, wk, wv, qn_w, kn_w, wo):
    import ml_dtypes
    from concourse.bass_utils import run_bass_kernel_spmd

    x = np.asarray(x, dtype=np.float32)
    norm_w = np.asarray(norm_w, dtype=np.float32)
    wq = np.asarray(wq, dtype=np.float32)
    wk = np.asarray(wk, dtype=np.float32)
    wv = np.asarray(wv, dtype=np.float32)
    qn_w = np.asarray(qn_w, dtype=np.float32)
    kn_w = np.asarray(kn_w, dtype=np.float32)
    wo = np.asarray(wo, dtype=np.float32)
    B = x.shape[0]

    nc = _get_nc()
    in_maps = []
    for c in range(8):
        b, g = c // 4, c % 4
        ms = slice(g * M_LOC, (g + 1) * M_LOC)
        gq = norm_w[:, None] * wq[:, ms]
        gk = norm_w[:, None] * wk[:, ms]
        gv = norm_w[:, None] * wv[:, ms]
        negc = -np.stack([gq.sum(0), gk.sum(0), gv.sum(0)])
        in_maps.append({
            "x_nat": np.ascontiguousarray(x[b]).astype(ml_dtypes.bfloat16),
            "x_tr": np.ascontiguousarray(x[b].T).astype(ml_dtypes.bfloat16),
            "wgq": np.ascontiguousarray(gq).astype(ml_dtypes.bfloat16),
            "wgk": np.ascontiguousarray(gk).astype(ml_dtypes.bfloat16),
            "wgv": np.ascontiguousarray(gv).astype(ml_dtypes.bfloat16),
            "negc": np.ascontiguousarray(negc).astype(ml_dtypes.bfloat16),
            "wo": np.ascontiguousarray(wo[ms, :]).astype(ml_dtypes.bfloat16),
            "qn_w": qn_w,
            "kn_w": kn_w,
        })
    res = run_bass_kernel_spmd(nc, in_maps, core_ids=list(range(8)))
    out = np.zeros((B, S, D), dtype=np.float32)
    for c in range(8):
        out[c // 4] += np.asarray(res.results[c]["out"], dtype=np.float32)
    return out


# revision 15
# speedup vs baseline: 1.2524x; 1.1338x over previous
"""Trainium2 Bass kernel for nn_Attention_55894704390617.

Dense transformer attention block:
  xn = LN(x) ; q,k,v = xn @ wq/wk/wv ; q,k = headLN(q),headLN(k)
  out = softmax(q k^T / sqrt(dh)) v @ wo

Sharding over 8 NeuronCores: 2 (batch) x 4 (head groups of 8 heads).
Each core computes a partial output (its head-group's contribution to
out = attn_out @ wo); the host sums the 4 partials per batch.

Per-core data flow (matmuls in bf16, fp32 PSUM accumulation):
  - host pre-folds norm_w into wq/wk/wv and ships the transposed x and
    the -colsum(w) correction rows; LN mean is folded into each
    projection as a K=1 accumulation row (mu[s] x negc[m]); x-rstd is
    applied only to V (head-LN on Q/K is scale-invariant, so their
    x-rstd cancels)
  - all transposes (kT, qT, avT) are DMA xbar transposes issued from
    the otherwise-idle SP engine: no PE transposes, no PSUM staging
  - scores computed transposed (scoresT[j,i]) two heads per PSUM pair
    [128,1024]; one exp per pair on ACT (phase B ACT does only exp)
  - attn@V uses attn tiles as the stationary operand and V (+ones col
    for the softmax denominator) as the 65-column moving operand:
    cost-model matmul time scales with moving columns only
  - AV lands natural [i, dh]; denominator normalize is a per-partition
    scalar multiply, then a DMA transpose produces avT for the
    out-projection
  - head-LN apply runs on GPSIMD; bn stats on DVE read PSUM directly
  - out partials are written in bf16
"""

import numpy as np

S = 2048          # sequence length
D = 2048          # model dim
H_LOC = 8         # heads per core
DH = 64           # head dim
M_LOC = H_LOC * DH  # 512 inner dim per core
N_D = D // 128    # 16 d-tiles
N_S = S // 128    # 16 s-tiles
N_SC = S // 512   # 4 512-chunks
N_MT = M_LOC // 128  # 4 m-tiles per core
EPS = 1e-5

_COMPILED = {}


def _build():
    from concourse._compat import axon_active
    axon_active()
    import concourse.bacc as bacc
    import concourse.mybir as mybir
    import concourse.tile as tile
    from contextlib import ExitStack

    F32 = mybir.dt.float32
    BF16 = mybir.dt.bfloat16
    AF = mybir.ActivationFunctionType
    OP = mybir.AluOpType

    nc = bacc.Bacc(None, target_bir_lowering=False)

    x_nat = nc.dram_tensor("x_nat", [S, D], BF16, kind="ExternalInput")
    x_tr = nc.dram_tensor("x_tr", [D, S], BF16, kind="ExternalInput")
    wgq = nc.dram_tensor("wgq", [D, M_LOC], BF16, kind="ExternalInput")
    wgk = nc.dram_tensor("wgk", [D, M_LOC], BF16, kind="ExternalInput")
    wgv = nc.dram_tensor("wgv", [D, M_LOC], BF16, kind="ExternalInput")
    negc = nc.dram_tensor("negc", [3, M_LOC], BF16, kind="ExternalInput")
    wo = nc.dram_tensor("wo", [M_LOC, D], BF16, kind="ExternalInput")
    qn_w = nc.dram_tensor("qn_w", [DH], F32, kind="ExternalInput")
    kn_w = nc.dram_tensor("kn_w", [DH], F32, kind="ExternalInput")
    out = nc.dram_tensor("out", [S, D], BF16, kind="ExternalOutput")

    import os
    DBG = bool(os.environ.get("ATTN_DEBUG"))
    dbg = {}
    if DBG:
        dbg["kT"] = nc.dram_tensor("dbg_kT", [128, N_MT, S], BF16,
                                   kind="ExternalOutput")
        dbg["vext0"] = nc.dram_tensor("dbg_vext0", [128, H_LOC, DH + 1], BF16,
                                      kind="ExternalOutput")
        dbg["qT0"] = nc.dram_tensor("dbg_qT0", [128, N_MT, 512], BF16,
                                    kind="ExternalOutput")
        dbg["avT0"] = nc.dram_tensor("dbg_avT0", [128, N_MT, 512], BF16,
                                     kind="ExternalOutput")
        dbg["attn00"] = nc.dram_tensor("dbg_attn00", [128, 1024], BF16,
                                       kind="ExternalOutput")

    with tile.TileContext(nc) as tc:
        es = ExitStack()
        # ---- pools alive for the whole kernel ----
        consts = es.enter_context(tc.tile_pool(name="consts", bufs=1))
        dram = es.enter_context(tc.tile_pool(name="dram", bufs=1, space="DRAM"))
        xt_pool = es.enter_context(tc.tile_pool(name="xt", bufs=1))
        wgq_pool = es.enter_context(tc.tile_pool(name="wgq", bufs=1))
        kT_pool = es.enter_context(tc.tile_pool(name="kT", bufs=1))
        vext_pool = es.enter_context(tc.tile_pool(name="vext", bufs=1))

        eps_t = consts.tile([128, 1], F32, name="eps_t")
        nc.vector.memset(eps_t, EPS)
        from concourse.masks import make_identity
        ident = consts.tile([128, 128], BF16, name="ident")
        make_identity(nc, ident)

        # qn/kn replicated across partitions (f32 dma, then bf16 copy)
        from concourse.bass import AP
        rep_f32 = {}
        rep_bf = {}
        for nm, wten in (("q", qn_w), ("k", kn_w)):
            rf = consts.tile([128, H_LOC, DH], F32, name=f"{nm}n_repf")
            bsrc = AP(tensor=wten[:].tensor, offset=wten[:].offset,
                      ap=[[0, 128], [0, H_LOC], [1, DH]])
            nc.scalar.dma_start(out=rf, in_=bsrc)
            rb = consts.tile([128, H_LOC * DH], BF16, name=f"{nm}n_rep")
            nc.vector.tensor_copy(rb, rf.rearrange("p h d -> p (h d)"))
            rep_f32[nm] = rf
            rep_bf[nm] = rb

        negc_sb = [consts.tile([1, M_LOC], BF16, name=f"negc_sb{r}")
                   for r in range(3)]
        for r in range(3):
            nc.scalar.dma_start(out=negc_sb[r], in_=negc[r:r + 1, :])

        # per-s-tile stat tiles
        mu_col = [consts.tile([128, 1], F32, name=f"mu_col{t}") for t in range(N_S)]
        rstd_col = [consts.tile([128, 1], F32, name=f"rstd_col{t}")
                    for t in range(N_S)]
        mu_bf = [consts.tile([1, 128], BF16, name=f"mu_bf{t}") for t in range(N_S)]

        xt = [xt_pool.tile([128, S], BF16, name=f"xt{t}") for t in range(N_D)]
        wg_q = [wgq_pool.tile([128, M_LOC], BF16, name=f"wg_q{t}")
                for t in range(N_D)]
        qT_pool = es.enter_context(tc.tile_pool(name="qT", bufs=2))
        kT = kT_pool.tile([128, N_MT, S], BF16, name="kT")
        v_ext = [vext_pool.tile([128, H_LOC, DH + 1], BF16, name=f"vext{st}")
                 for st in range(N_S)]

        # ============ phase A: loads + stats + K,V projections ============
        with ExitStack() as ph:
            wg_pool = ph.enter_context(tc.tile_pool(name="wg", bufs=1))
            stage = ph.enter_context(tc.tile_pool(name="stage", bufs=3))
            scrA = ph.enter_context(tc.tile_pool(name="scrA", bufs=3))
            ps_mm = ph.enter_context(tc.tile_pool(name="ps_mm", bufs=5, space="PSUM"))
            ps_mu = ph.enter_context(tc.tile_pool(name="ps_mu", bufs=2,
                                                  space="PSUM"))

            wg = {"q": wg_q}
            for wname in ("k", "v"):
                wg[wname] = [wg_pool.tile([128, M_LOC], BF16, name=f"wg_{wname}{t}")
                             for t in range(N_D)]
            wdrams = {"q": wgq, "k": wgk, "v": wgv}
            NEGC_ROW = {"q": 0, "k": 1, "v": 2}
            qT0 = qT_pool.tile([128, N_MT, 512], BF16, tag="qT", name="qT0")

            def emit_stats(st):
                # token mean/var: even s-tiles via ACT accumulators, odd via
                # DVE bn_stats — splits the work across both engines
                xst = stage.tile([128, S], BF16, tag="xst")
                nc.scalar.dma_start(out=xst,
                                    in_=x_nat[st * 128:(st + 1) * 128, :])
                var = scrA.tile([128, 1], F32, tag="var")
                if st % 2 == 0:
                    scr = stage.tile([128, S], BF16, tag="scr")
                    s1 = scrA.tile([128, 1], F32, tag="s1")
                    s2 = scrA.tile([128, 1], F32, tag="s2")
                    nc.scalar.activation(out=scr, in_=xst, func=AF.Copy,
                                         accum_out=s1)
                    nc.scalar.activation(out=scr, in_=xst, func=AF.Square,
                                         accum_out=s2)
                    nc.vector.tensor_scalar_mul(out=mu_col[st], in0=s1,
                                                scalar1=1.0 / D)
                    nc.vector.tensor_scalar_mul(out=var, in0=s2,
                                                scalar1=1.0 / D)
                    mu2 = scrA.tile([128, 1], F32, tag="mu2")
                    nc.vector.tensor_mul(out=mu2, in0=mu_col[st],
                                         in1=mu_col[st])
                    nc.vector.tensor_sub(out=var, in0=var, in1=mu2)
                else:
                    xg = xst.rearrange("p (n f) -> p n f", f=512)
                    bn = scrA.tile([128, 4, 6], F32, tag="bn")
                    for sg in range(4):
                        nc.vector.bn_stats(out=bn[:, sg, :], in_=xg[:, sg, :])
                    mv = scrA.tile([128, 2], F32, tag="mv")
                    nc.vector.bn_aggr(out=mv, in_=bn)
                    nc.vector.tensor_copy(mu_col[st], mv[:, 0:1])
                    nc.vector.tensor_copy(var, mv[:, 1:2])
                mu_cb = scrA.tile([128, 1], BF16, tag="mu_cb")
                nc.vector.tensor_copy(mu_cb, mu_col[st])
                mu_ps = ps_mu.tile([1, 128], F32, tag="mu",
                                   name=f"mu_ps{st}")
                nc.tensor.matmul(mu_ps[:, :], mu_cb[:, :], ident[:, :],
                                 start=True, stop=True)
                nc.vector.tensor_copy(mu_bf[st], mu_ps)
                # rstd = exp(-0.5*ln(var+eps)): Ln/Exp share one ACT table
                # with the softmax Exp, so no table reloads
                nc.scalar.activation(out=rstd_col[st], in_=var,
                                     func=AF.Ln, bias=eps_t, scale=1.0)
                nc.scalar.activation(out=rstd_col[st], in_=rstd_col[st],
                                     func=AF.Exp, scale=-0.5)
            # input DMAs: x on SP, weights on ACT (idle in phase A)
            for t in range(N_D):
                nc.sync.dma_start(out=wg["k"][t],
                                  in_=wgk[t * 128:(t + 1) * 128, :])
                nc.sync.dma_start(out=xt[t], in_=x_tr[t * 128:(t + 1) * 128, :])
                nc.scalar.dma_start(out=wg["v"][t],
                                    in_=wgv[t * 128:(t + 1) * 128, :])
                emit_stats(t)
            # Q weights: land while K/V projections run
            for t in range(N_D):
                nc.sync.dma_start(out=wg_q[t],
                                  in_=wgq[t * 128:(t + 1) * 128, :])

            def emit_headln_tail(p, dst_T, sl, nm):
                """Head-LN on PSUM proj result p, then DMA-transpose into
                dst_T[:, :, sl]. nm selects qn/kn."""
                pg = p.rearrange("p (h d) -> p h d", d=DH)
                bn8 = scrA.tile([128, H_LOC, 6], F32, tag="bn8")
                mv8 = scrA.tile([128, H_LOC, 2], F32, tag="mv8")
                for h in range(H_LOC):
                    nc.vector.bn_stats(out=bn8[:, h, :], in_=pg[:, h, :])
                    nc.vector.bn_aggr(out=mv8[:, h, :], in_=bn8[:, h, :])
                rstd8 = scrA.tile([128, H_LOC], F32, tag="rstd8")
                nc.scalar.activation(out=rstd8, in_=mv8[:, :, 1], func=AF.Ln,
                                     bias=eps_t, scale=1.0)
                nc.scalar.activation(out=rstd8, in_=rstd8, func=AF.Exp,
                                     scale=-0.5)
                lnb = scrA.tile([128, H_LOC, DH], BF16, tag="lnb")
                for h in range(H_LOC):
                    nc.vector.tensor_scalar(
                        out=lnb[:, h, :], in0=pg[:, h, :],
                        scalar1=mv8[:, h, 0:1], scalar2=rstd8[:, h:h + 1],
                        op0=OP.subtract, op1=OP.mult)
                lnb2 = scrA.tile([128, H_LOC * DH], BF16, tag="lnb2")
                nc.gpsimd.tensor_mul(out=lnb2,
                                     in0=lnb.rearrange("p h d -> p (h d)"),
                                     in1=rep_bf[nm])
                nc.sync.dma_start_transpose(out=dst_T[:, :, sl], in_=lnb2)

            for st in range(N_S):
                sl = slice(st * 128, (st + 1) * 128)
                # first-chunk Q projections ride along mid-phase (their
                # weights have landed by then); qT0 is ready well before
                # phase B's first scores
                if 8 <= st < 12:
                    q0 = st - 8
                    qsl0 = slice(q0 * 128, (q0 + 1) * 128)
                    p3 = ps_mm.tile([128, M_LOC], F32, tag="mm",
                                    name=f"pq{q0}")
                    for t in range(N_D):
                        nc.tensor.matmul(p3[:, :], xt[t][:, qsl0],
                                         wg_q[t][:, :],
                                         start=(t == 0), stop=False)
                    nc.tensor.matmul(p3[:, :], mu_bf[q0][:, :],
                                     negc_sb[NEGC_ROW["q"]][:, :],
                                     start=False, stop=True)
                    emit_headln_tail(
                        p3, qT0, slice(q0 * 128, (q0 + 1) * 128), "q")
                # K projection + head-LN + transpose
                p = ps_mm.tile([128, M_LOC], F32, tag="mm", name=f"pk{st}")
                for t in range(N_D):
                    nc.tensor.matmul(p[:, :], xt[t][:, sl], wg["k"][t][:, :],
                                     start=(t == 0), stop=False)
                nc.tensor.matmul(p[:, :], mu_bf[st][:, :],
                                 negc_sb[NEGC_ROW["k"]][:, :],
                                 start=False, stop=True)
                emit_headln_tail(p, kT, sl, "k")
                # V projection + x-rstd + ones column
                p2 = ps_mm.tile([128, M_LOC], F32, tag="mm", name=f"pv{st}")
                for t in range(N_D):
                    nc.tensor.matmul(p2[:, :], xt[t][:, sl], wg["v"][t][:, :],
                                     start=(t == 0), stop=False)
                nc.tensor.matmul(p2[:, :], mu_bf[st][:, :],
                                 negc_sb[NEGC_ROW["v"]][:, :],
                                 start=False, stop=True)
                nc.vector.tensor_scalar_mul(
                    out=v_ext[st][:, :, 0:DH],
                    in0=p2.rearrange("p (h d) -> p h d", d=DH),
                    scalar1=rstd_col[st])
                nc.vector.memset(v_ext[st][:, :, DH:DH + 1], 1.0)

        if DBG:
            nc.sync.dma_start(out=dbg["kT"][:, :, :], in_=kT)
            nc.sync.dma_start(out=dbg["vext0"][:, :, :], in_=v_ext[0])

        # ============ phase B: Q proj + attention + out-proj ============
        with ExitStack() as ph:
            wo_pool = ph.enter_context(tc.tile_pool(name="wop", bufs=1))
            attn_pool = ph.enter_context(tc.tile_pool(name="attn", bufs=12))
            avsb_pool = ph.enter_context(tc.tile_pool(name="avsb", bufs=2))
            avT_pool = ph.enter_context(tc.tile_pool(name="avT", bufs=2))
            ot_pool = ph.enter_context(tc.tile_pool(name="ot", bufs=3))
            scrB = ph.enter_context(tc.tile_pool(name="scrB", bufs=3))
            # PSUM: sc 2x2 banks + av 2x1 + qp 1 + op 1 = 8 banks
            ps_sc = ph.enter_context(tc.tile_pool(name="ps_sc", bufs=2,
                                                  space="PSUM"))
            ps_av = ph.enter_context(tc.tile_pool(name="ps_av", bufs=2,
                                                  space="PSUM"))
            ps_qp = ph.enter_context(tc.tile_pool(name="ps_qp", bufs=1,
                                                  space="PSUM"))
            ps_op = ph.enter_context(tc.tile_pool(name="ps_op", bufs=1,
                                                  space="PSUM"))

            wo_sb = [wo_pool.tile([128, D], BF16, name=f"wo{mt}")
                     for mt in range(N_MT)]
            for mt in range(N_MT):
                nc.scalar.dma_start(out=wo_sb[mt],
                                    in_=wo[mt * 128:(mt + 1) * 128, :])

            def emit_qtail(st, p, qT_next):
                ssl = slice((st % 4) * 128, (st % 4 + 1) * 128)
                nc.tensor.matmul(p[:, :], mu_bf[st][:, :],
                                 negc_sb[NEGC_ROW["q"]][:, :],
                                 start=False, stop=True)
                pg = p.rearrange("p (h d) -> p h d", d=DH)
                bn8 = scrB.tile([128, H_LOC, 6], F32, tag="bn8")
                mv8 = scrB.tile([128, H_LOC, 2], F32, tag="mv8")
                for h in range(H_LOC):
                    nc.vector.bn_stats(out=bn8[:, h, :], in_=pg[:, h, :])
                    nc.vector.bn_aggr(out=mv8[:, h, :], in_=bn8[:, h, :])
                rstd8 = scrB.tile([128, H_LOC], F32, tag="rstd8")
                nc.scalar.activation(out=rstd8, in_=mv8[:, :, 1], func=AF.Ln,
                                     bias=eps_t, scale=1.0)
                nc.scalar.activation(out=rstd8, in_=rstd8, func=AF.Exp,
                                     scale=-0.5)
                lnb = scrB.tile([128, H_LOC, DH], BF16, tag="lnb")
                for h in range(H_LOC):
                    nc.vector.tensor_scalar(
                        out=lnb[:, h, :], in0=pg[:, h, :],
                        scalar1=mv8[:, h, 0:1], scalar2=rstd8[:, h:h + 1],
                        op0=OP.subtract, op1=OP.mult)
                lnb2 = scrB.tile([128, H_LOC * DH], BF16, tag="lnb2")
                nc.vector.tensor_mul(out=lnb2,
                                     in0=lnb.rearrange("p h d -> p (h d)"),
                                     in1=rep_bf["q"])
                nc.sync.dma_start_transpose(out=qT_next[:, :, ssl], in_=lnb2)

            def emit_oproj(avT_src, st, do):
                sl = slice(st * 128, (st + 1) * 128)
                lsl = slice((st % 4) * 128, (st % 4 + 1) * 128)
                op = ps_op.tile([128, 512], F32, tag="op", name=f"op{st}_{do}")
                for mt in range(N_MT):
                    nc.tensor.matmul(op[:, :], avT_src[:, mt, lsl],
                                     wo_sb[mt][:, do * 512:(do + 1) * 512],
                                     start=(mt == 0), stop=(mt == N_MT - 1))
                ot = ot_pool.tile([128, 512], BF16, tag="ot")
                nc.vector.tensor_copy(ot, op)
                nc.gpsimd.dma_start(out=out[sl, do * 512:(do + 1) * 512],
                                    in_=ot)

            def alloc_qT(ic):
                return qT_pool.tile([128, N_MT, 512], BF16, tag="qT",
                                    name=f"qT{ic}")

            qT_cur = qT0
            avT_prev = None
            for ic in range(N_SC):
                qT_next = alloc_qT(ic + 1) if ic + 1 < N_SC else None
                avT_cur = avT_pool.tile([128, N_MT, 512], BF16, tag="avT",
                                        name=f"avT{ic}")
                for hp in range(N_MT):
                    qst = 4 * (ic + 1) + hp if qT_next is not None else None
                    qp = None
                    if qst is not None:
                        qp = ps_qp.tile([128, M_LOC], F32, tag="qp",
                                        name=f"qp{qst}")
                    avsb = avsb_pool.tile([128, 4, 2, DH], BF16, tag="avsb",
                                          name=f"avsb{ic}_{hp}")

                    def emit_av_chain(attn_tiles, hs, it, h):
                        # one 16-step accumulation chain in its own PSUM bank
                        # (in-flight chains sharing a bank corrupt each other),
                        # then denominator normalize into avsb
                        av = ps_av.tile([128, DH + 1], F32, tag="av",
                                        name=f"av{ic}_{hp}_{hs}_{it}")
                        for pj in range(8):
                            for half in range(2):
                                csl = slice(half * 512 + it * 128,
                                            half * 512 + (it + 1) * 128)
                                nc.tensor.matmul(
                                    av[:, 0:DH + 1], attn_tiles[pj][:, csl],
                                    v_ext[2 * pj + half][:, h, :],
                                    start=(pj == 0 and half == 0),
                                    stop=(pj == 7 and half == 1))
                        nc.vector.reciprocal(av[:, DH:DH + 1], av[:, DH:DH + 1])
                        nc.vector.tensor_scalar_mul(
                            out=avsb[:, it, hs, :], in0=av[:, 0:DH],
                            scalar1=av[:, DH:DH + 1])

                    qp_step = [0]

                    def emit_qp_step(n=2):
                        # front-loaded: the whole chain lands during hs0 so
                        # the qtail + qT transpose finish mid-hp, hiding the
                        # DMA-transpose latency from the next chunk's scores
                        for _ in range(n):
                            if qp is not None and qp_step[0] < N_D:
                                t = qp_step[0]
                                qsl = slice(qst * 128, (qst + 1) * 128)
                                nc.tensor.matmul(qp[:, :], xt[t][:, qsl],
                                                 wg_q[t][:, :],
                                                 start=(t == 0), stop=False)
                                qp_step[0] += 1

                    for hs in range(2):
                        h = 2 * hp + hs
                        psl = slice(hs * DH, (hs + 1) * DH)
                        attn_tiles = []
                        for pj in range(8):
                            if avT_prev is not None and pj % 4 == 1:
                                emit_oproj(avT_prev, 4 * (ic - 1) + hp,
                                           2 * hs + pj // 4)
                            sc = ps_sc.tile([128, 1024], F32, tag="sc",
                                            name=f"sc{hs}_{pj}")
                            for half in range(2):
                                jt = 2 * pj + half
                                jsl = slice(jt * 128, (jt + 1) * 128)
                                nc.tensor.matmul(
                                    sc[:, half * 512:(half + 1) * 512],
                                    kT[psl, hp, jsl], qT_cur[psl, hp, :],
                                    start=True, stop=True)
                            a = attn_pool.tile([128, 1024], BF16, tag="attn")
                            nc.scalar.activation(out=a, in_=sc, func=AF.Exp,
                                                 scale=0.125)
                            if DBG and ic == 0 and hp == 0 and hs == 0 \
                                    and pj == 0:
                                nc.sync.dma_start(out=dbg["attn00"][:, :],
                                                  in_=a)
                            attn_tiles.append(a)
                            emit_qp_step()
                        # pass 1: chains for i-blocks 0,1 (banks A,B)
                        for it in range(2):
                            emit_av_chain(attn_tiles, hs, it, h)
                        # pass 2: chains for i-blocks 2,3 reuse the banks
                        for it in range(2, 4):
                            emit_av_chain(attn_tiles, hs, it, h)
                        if hs == 0 and qp is not None:
                            emit_qtail(qst, qp, qT_next)
                    nc.sync.dma_start_transpose(
                        out=avT_cur[:, hp, :].rearrange(
                            "p (t f) -> p t f", f=128),
                        in_=avsb.rearrange("p t h d -> p (t h d)"))
                if DBG and ic == 0:
                    nc.sync.dma_start(out=dbg["qT0"][:, :, :], in_=qT_cur)
                    nc.sync.dma_start(out=dbg["avT0"][:, :, :], in_=avT_cur)
                avT_prev = avT_cur
                qT_cur = qT_next

            # tail: out-projection of the last chunk
            for st in range(4 * (N_SC - 1), 4 * N_SC):
                for do in range(N_SC):
                    emit_oproj(avT_prev, st, do)
        es.close()

    # The act-table-load inserter picks the first table containing each
    # activation's function; Exp appears in three tables while Ln is only
    # in natural_log_exp_and_others, which makes Exp<->Ln alternation
    # reload tables constantly. Prune Exp from the other tables (set ids
    # keep their positions) so every function resolves to the shared one
    # and the table is loaded exactly once.
    import concourse.bacc as bacc_mod
    orig_tables = bacc_mod.get_activation_tables

    def pruned_tables(arch):
        tabs = orig_tables(arch)
        return {
            name: (s if name == "natural_log_exp_and_others"
                   else s - {AF.Exp, AF.Square, AF.Copy, AF.Identity})
            for name, s in tabs.items()
        }

    bacc_mod.get_activation_tables = pruned_tables
    try:
        nc.compile()
    finally:
        bacc_mod.get_activation_tables = orig_tables
    return nc


def _get_nc():
    if "nc" not in _COMPILED:
        _COMPILED["nc"] = _build()
    return _COMPILED["nc"]


def kernel(x, norm_w, wq, wk, wv, qn_w, kn_w, wo):
    import ml_dtypes
    from concourse.bass_utils import run_bass_kernel_spmd

    x = np.asarray(x, dtype=np.float32)
    norm_w = np.asarray(norm_w, dtype=np.float32)
    wq = np.asarray(wq, dtype=np.float32)
    wk = np.asarray(wk, dtype=np.float32)
    wv = np.asarray(wv, dtype=np.float32)
    qn_w = np.asarray(qn_w, dtype=np.float32)
    kn_w = np.asarray(kn_w, dtype=np.float32)
    wo = np.asarray(wo, dtype=np.float32)
    B = x.shape[0]

    nc = _get_nc()
    in_maps = []
    for c in range(8):
        b, g = c // 4, c % 4
        ms = slice(g * M_LOC, (g + 1) * M_LOC)
        gq = norm_w[:, None] * wq[:, ms]
        gk = norm_w[:, None] * wk[:, ms]
        gv = norm_w[:, None] * wv[:, ms]
        negc = -np.stack([gq.sum(0), gk.sum(0), gv.sum(0)])
        in_maps.append({
            "x_nat": np.ascontiguousarray(x[b]).astype(ml_dtypes.bfloat16),
            "x_tr": np.ascontiguousarray(x[b].T).astype(ml_dtypes.bfloat16),
            "wgq": np.ascontiguousarray(gq).astype(ml_dtypes.bfloat16),
            "wgk": np.ascontiguousarray(gk).astype(ml_dtypes.bfloat16),
            "wgv": np.ascontiguousarray(gv).astype(ml_dtypes.bfloat16),
            "negc": np.ascontiguousarray(negc).astype(ml_dtypes.bfloat16),
            "wo": np.ascontiguousarray(wo[ms, :]).astype(ml_dtypes.bfloat16),
            "qn_w": qn_w,
            "kn_w": kn_w,
        })
    res = run_bass_kernel_spmd(nc, in_maps, core_ids=list(range(8)))
    out = np.zeros((B, S, D), dtype=np.float32)
    for c in range(8):
        out[c // 4] += np.asarray(res.results[c]["out"], dtype=np.float32)
    return out
